# revision 24
# baseline (speedup 1.0000x reference)
"""Trainium2 Bass kernel for nn_MultiHeadAttention_53463752900838.

Math (per batch element b, one NeuronCore each — pure data parallel over B=8):
  qkv = w_qkv @ x + b_qkv                     (3072, T)
  q,k,v per head h: (64, T);  q scaled by 1/8 (folded into weights on host)
  scores[t,h,g] = sum_d q[h,d,t] k[g,d,t]     per-timestep 16x16 Gram matrix
  attn = softmax over t  (per (h,g) pair)
  context[h,d,t] = sum_g attn[t,h,g] v[g,d,t]
  out = w_out @ context + b_out               (1024, T)

Kernel layout (all bf16 matmuls, fp32 PSUM accumulation), software-pipelined
so the PE never idles:

  Pass 1 (per 256-t span s): project QKV; the PSUM evacuation adds b_qkv
    (activation Identity-with-bias / tensor_scalar add) and writes Q/K
    DIRECTLY into the scores layout qt/kt (64d, (h,t)) via two half-height
    evacs; V goes to a stage tile and is spilled+marshalled to DRAM in one
    strided DMA. Scores for span s-1 are emitted after the QKV matmuls of
    span s (PE queue stays full while evacs/DMAs of s-1 land); fused exp on
    ScalarE; running Z-reduce is delayed by 2 spans. exp(S) spills to DRAM.

  Pass 2 (per 512-t block): reload exp(S), normalize by 1/Z in place,
    per-t context matmuls with tile_position column tiling, one strided DMA
    re-marshals context to channel-major (cnat), final projection emitted one
    block behind so it overlaps the next block's context matmuls. Output is
    stored bf16 as out^T (t, o); host transposes and adds b_out.
"""

import os
import sys
import contextlib

import numpy as np
import ml_dtypes

for p in ("/opt/trn_rl_repo",):
    if p not in sys.path and os.path.isdir(p):
        sys.path.insert(0, p)

import concourse.bass as bass
import concourse.tile as tile
from concourse import mybir
from concourse.bass_utils import run_bass_kernel_spmd

F32 = mybir.dt.float32
BF16 = mybir.dt.bfloat16

N_CORES = 8
C = 1024
H = 16
DK = 64
OC3 = 3072


_WAITS2_OK = {
    "InstMatmult",
    "InstLdweights",
    "InstTensorCopy",
    "InstActivation",
    "InstTensorTensor",
    "InstTensorReduce",
    "InstDMACopy",
    "InstTensorScalarPtr",
    "InstMemset",
}


def _split_sync_waits(nc, limit=1):
    """walrus codegen rejects too many semaphore waits per instruction (CTRL
    class takes 1); hoist overflow waits onto NoOps inserted before the
    offending instruction. Compute/DMA instructions take 2."""
    counter = [0]
    n_split = 0
    for fn in nc.m.functions:
        for bb in fn.blocks:
            out = []
            for ins in bb.instructions:
                si = getattr(ins, "sync_info", None)
                waits = list(si.on_wait) if (si is not None and si.on_wait) else []
                if len(waits) > limit:
                    n_split += 1
                    extra, keep = waits[:-limit], waits[-limit:]
                    for i in range(0, len(extra), limit):
                        counter[0] += 1
                        out.append(
                            mybir.InstNoOp(
                                name=f"I-wsplit-{counter[0]}",
                                opcode="NoOp",
                                engine=ins.engine,
                                ins=[],
                                outs=[],
                                sync_info=mybir.SyncInfo(
                                    on_wait=list(extra[i : i + limit]), on_update=[]
                                ),
                            )
                        )
                    si.on_wait = keep
                out.append(ins)
            bb.instructions[:] = out
    return n_split


def build_kernel(T=4096, SPAN=256):
    """SPAN is kept for interface compat; pass-1 uses 512-t spans and pass-2
    uses 1024-t blocks of four 256-t sub-spans internally."""
    SP1 = 512  # pass-1 span
    NSP1 = T // SP1
    SS = 256  # pass-2 sub-span
    NW = 4  # sub-spans per pass-2 block
    SPC = NW * SS
    NBLK = T // SPC
    NSS = T // SS
    nc = bass.Bass("TRN2", target_bir_lowering=False, debug=False)

    x_in = nc.dram_tensor("x", [C, T], BF16, kind="ExternalInput").ap()
    wq_in = nc.dram_tensor("wqT", [C, OC3], BF16, kind="ExternalInput").ap()
    bq_in = nc.dram_tensor("bqc", [128, 24], F32, kind="ExternalInput").ap()
    wo_in = nc.dram_tensor("woT", [C, C], BF16, kind="ExternalInput").ap()
    out_t = nc.dram_tensor("outT", [T, C], BF16, kind="ExternalOutput").ap()
    # DRAM scratch: exp(scores) as (g, (h, t_abs)) and V as (g, (d, t_abs))
    se_d = nc.dram_tensor("se_d", [16, H * T], BF16).ap()
    vt_d = nc.dram_tensor("vt_d", [16, DK * T], BF16).ap()
    sev_d = se_d.rearrange("g (h t) -> g h t", h=H)
    vtv_d = vt_d.rearrange("(m hl) (d t) -> hl d m t", m=8, hl=2, d=DK)
    vbv_d = vt_d.rearrange("g (d t) -> g d t", d=DK)

    Exp = mybir.ActivationFunctionType.Exp
    Copy = mybir.ActivationFunctionType.Copy
    Ident = mybir.ActivationFunctionType.Identity
    ADD = mybir.AluOpType.add
    MUL = mybir.AluOpType.mult

    with tile.TileContext(nc) as tc, contextlib.ExitStack() as octx:
        const = octx.enter_context(tc.tile_pool(name="const", bufs=1))
        bqc = const.tile([128, 24], F32, tag="bqc")
        zacc = const.tile([16, 16], F32, tag="zacc")
        rrec = const.tile([16, 16], F32, tag="rrec")
        rrecb = const.tile([16, 16], BF16, tag="rrecb")
        wopool = octx.enter_context(tc.tile_pool(name="wo", bufs=1))
        wo_sb = []

        def emit_wo_chunk(k):
            w = wopool.tile([128, C], BF16, tag=f"wo{k}", name=f"wo{k}")
            nc.gpsimd.dma_start(w[:], wo_in[k * 128 : (k + 1) * 128, :])
            wo_sb.append(w)

        # ---------------- PASS 1 ----------------
        with contextlib.ExitStack() as ctx:
            wpool = ctx.enter_context(tc.tile_pool(name="wq", bufs=1))
            xpool = ctx.enter_context(tc.tile_pool(name="x", bufs=2))
            stpool = ctx.enter_context(tc.tile_pool(name="stage", bufs=2))
            qkpool = ctx.enter_context(tc.tile_pool(name="qkt", bufs=2))
            sepool = ctx.enter_context(tc.tile_pool(name="se", bufs=2))
            zpool = ctx.enter_context(tc.tile_pool(name="zp", bufs=2))
            ps_a = ctx.enter_context(tc.tile_pool(name="psA", bufs=4, space="PSUM"))
            ps_s = ctx.enter_context(tc.tile_pool(name="psS", bufs=3, space="PSUM"))

            xs = {}

            def emit_xload(s):
                xk = xpool.tile([128, 8 * SP1], BF16, tag="x")
                nc.sync.dma_start(
                    xk[:].rearrange("p (k t) -> p k t", k=8),
                    x_in[:, s * SP1 : (s + 1) * SP1].rearrange(
                        "(k p) t -> p k t", k=8
                    ),
                )
                xs[s] = xk

            # x span 0 + bias first so PE can start ASAP; wq chunks follow and
            # the inner-k matmul order paces with their arrival.
            nc.sync.dma_start(bqc[:], bq_in)
            emit_xload(0)
            wq_sb = []
            for k in range(8):
                w = wpool.tile([128, OC3], BF16, tag=f"wq{k}")
                nc.sync.dma_start(w[:], wq_in[k * 128 : (k + 1) * 128, :])
                wq_sb.append(w)

            qts, kts, ses = {}, {}, {}
            eng_tog = [0]

            def evac(dst, src, bias):
                """PSUM->SBUF evacuation with bias add, alternating engines."""
                eng_tog[0] ^= 1
                if eng_tog[0]:
                    nc.scalar.activation(dst, src, Ident, bias=bias)
                else:
                    nc.vector.tensor_scalar(dst, src, bias, None, ADD)

            NBLK1 = SP1 // 32  # scores blocks per span

            def emit_scores_block(s, blk, partial_zred=False):
                """One 32-t scores block (Gram matmuls + fused exp) of span s.
                On blk==0 allocates the span's se tile; on the last block
                spills exp(S) to DRAM."""
                if blk == 0:
                    ses[s] = sepool.tile([16, H * SP1], BF16, tag="se", name="se")
                se = ses[s]
                qtv = qts[s][:].rearrange("p (h t) -> p t h", h=H)
                ktv = kts[s][:].rearrange("p (g t) -> p t g", g=H)
                sev = se[:].rearrange("p (h t) -> p t h", h=H)
                pss = ps_s.tile([16, 512], F32, tag="psS")
                for s32 in range(32):
                    tl = blk * 32 + s32
                    nc.tensor.matmul(
                        pss[:, s32 * 16 : (s32 + 1) * 16],
                        lhsT=ktv[:, tl, :],
                        rhs=qtv[:, tl, :],
                        start=True,
                        stop=True,
                    )
                nc.scalar.activation(
                    sev[:, blk * 32 : (blk + 1) * 32, :],
                    pss[:].rearrange("p (t h) -> p t h", h=H),
                    Exp,
                )
                if partial_zred:
                    zp = zpool.tile([16, 16], F32, tag="zp")
                    nc.vector.tensor_reduce(
                        zp[:],
                        se[:].rearrange("p (h t) -> p h t", h=H)[
                            :, :, blk * 32 : (blk + 1) * 32
                        ],
                        axis=mybir.AxisListType.X,
                        op=ADD,
                    )
                    nc.vector.tensor_tensor(
                        out=zacc[:], in0=zacc[:], in1=zp[:], op=ADD
                    )
                if blk == NBLK1 - 1:
                    qts.pop(s)
                    kts.pop(s)
                    nc.gpsimd.dma_start(
                        sev_d[:, :, s * SP1 : (s + 1) * SP1],
                        se[:].rearrange("p (h t) -> p h t", h=H),
                    )

            def emit_qkv(s, sc=None):
                """QKV projection of span s; scores blocks of span sc (if any)
                are interleaved between the m-tiles so the PE never waits for
                the Act-paced exp evacuations."""
                xall = xs.pop(s)
                qt = qkpool.tile([64, H * SP1], BF16, tag="qt")
                kt = qkpool.tile([64, H * SP1], BF16, tag="kt")
                stage = stpool.tile([128, 8 * SP1], BF16, tag="st")
                qts[s], kts[s] = qt, kt
                for m in range(24):
                    kind, mm = divmod(m, 8)
                    ps = ps_a.tile([128, SP1], F32, tag="psA")
                    for k in range(8):
                        nc.tensor.matmul(
                            ps[:],
                            lhsT=wq_sb[k][:, m * 128 : (m + 1) * 128],
                            rhs=xall[:, k * SP1 : (k + 1) * SP1],
                            start=(k == 0),
                            stop=(k == 7),
                        )
                    if kind < 2:
                        dstt = qt if kind == 0 else kt
                        for hl in range(2):
                            h_abs = 2 * mm + hl
                            evac(
                                dstt[:, h_abs * SP1 : (h_abs + 1) * SP1],
                                ps[hl * 64 : (hl + 1) * 64, :],
                                bqc[hl * 64 : (hl + 1) * 64, m : m + 1],
                            )
                    else:
                        evac(
                            stage[:, mm * SP1 : (mm + 1) * SP1],
                            ps[:],
                            bqc[:, m : m + 1],
                        )
                    if sc is not None and m < NBLK1:
                        emit_scores_block(sc, m)
                # V spill+marshal: two strided DMAs,
                # SBUF (hl*64+d, (m,t)) -> DRAM (g=2m+hl, (d, t_abs))
                for hl in range(2):
                    nc.gpsimd.dma_start(
                        vtv_d[hl, :, :, s * SP1 : (s + 1) * SP1],
                        stage[hl * 64 : (hl + 1) * 64, :].rearrange(
                            "d (m t) -> d m t", m=8
                        ),
                    )

            def emit_zred(s):
                zp = zpool.tile([16, 16], F32, tag="zp")
                nc.vector.tensor_reduce(
                    zp[:],
                    ses.pop(s)[:].rearrange("p (h t) -> p h t", h=H),
                    axis=mybir.AxisListType.X,
                    op=ADD,
                )
                if s == 0:
                    nc.vector.tensor_copy(zacc[:], zp[:])
                else:
                    nc.vector.tensor_tensor(out=zacc[:], in0=zacc[:], in1=zp[:], op=ADD)

            for s in range(NSP1):
                emit_qkv(s, sc=s - 1 if s >= 1 else None)
                if s + 1 < NSP1:
                    emit_xload(s + 1)
                if s >= 2:
                    emit_zred(s - 2)
                # prefetch the pass-2 projection weights on the idle DMA
                # window mid-pass-1 (gpsimd queue; no WAR entanglement)
                if 2 <= s < 6:
                    emit_wo_chunk(2 * (s - 2))
                    emit_wo_chunk(2 * (s - 2) + 1)
            # last span's scores: emitted straight, with per-block partial
            # Z-reduces so the softmax denominator is ready ASAP after the
            # final exp (shortens the pass-1 -> pass-2 transition).
            emit_zred(NSP1 - 2)
            ls = NSP1 - 1
            for blk in range(NBLK1):
                emit_scores_block(ls, blk, partial_zred=True)
            ses.pop(ls)
            nc.vector.reciprocal(rrec[:], zacc[:])
            nc.vector.tensor_copy(rrecb[:], rrec[:])

        # ---------------- PASS 2 ----------------
        with contextlib.ExitStack() as ctx:
            sebpool = ctx.enter_context(tc.tile_pool(name="seb", bufs=2))
            atpool = ctx.enter_context(tc.tile_pool(name="atn", bufs=4))
            vtpool = ctx.enter_context(tc.tile_pool(name="vt2", bufs=2))
            cpool = ctx.enter_context(tc.tile_pool(name="csb", bufs=2))
            cnpool = ctx.enter_context(tc.tile_pool(name="cnat", bufs=2))
            opool = ctx.enter_context(tc.tile_pool(name="osb", bufs=2))
            ps_c = ctx.enter_context(tc.tile_pool(name="psC", bufs=4, space="PSUM"))
            ps_o = ctx.enter_context(tc.tile_pool(name="psO", bufs=3, space="PSUM"))

            rbc = rrecb[:].unsqueeze(2).broadcast_to([16, 16, SS])
            eng2 = [0]

            def evac2(dst, src):
                eng2[0] ^= 1
                if eng2[0]:
                    nc.scalar.activation(dst, src, Copy)
                else:
                    nc.vector.tensor_copy(dst, src)

            norm = {}

            def emit_norm(si):
                """Load + normalize exp(S) for 256-t sub-span si (prefetched).
                The normalize writes a separate tile (not in-place) so the
                load WAR chain does not couple into the multiply's latency,
                and alternates DVE / GpSimd so neither engine serializes."""
                seb = sebpool.tile([16, H * SS], BF16, tag="seb")
                nc.gpsimd.dma_start(
                    seb[:].rearrange("p (h t) -> p h t", h=H),
                    sev_d[:, :, si * SS : (si + 1) * SS],
                )
                atn = atpool.tile([16, H * SS], BF16, tag="atn")
                eng = nc.gpsimd if si % 3 == 2 else nc.vector
                eng.tensor_tensor(
                    out=atn[:].rearrange("p (h t) -> p h t", h=H),
                    in0=seb[:].rearrange("p (h t) -> p h t", h=H),
                    in1=rbc,
                    op=MUL,
                )
                norm[si] = atn

            vts = {}

            def emit_vtload(si):
                vt = vtpool.tile([16, DK * SS], BF16, tag="vt2")
                nc.gpsimd.dma_start(
                    vt[:].rearrange("p (d t) -> p d t", d=DK),
                    vbv_d[:, :, si * SS : (si + 1) * SS],
                )
                vts[si] = vt

            cnats = {}

            def emit_ctx(b):
                cnats[b] = cnpool.tile([128, 8 * SPC], BF16, tag="cnat", name="cnat")
                csb = None
                for w in range(NW):
                    if w % 2 == 0:
                        csb = cpool.tile([128, DK * 128], BF16, tag="csb", name="csb")
                    si = b * NW + w
                    if si + 3 < NSS:
                        emit_norm(si + 3)
                    if si + 1 < NSS:
                        emit_vtload(si + 1)
                    atv = norm.pop(si)[:].rearrange("p (h t) -> p t h", h=H)
                    vtv = vts.pop(si)[:].rearrange("p (d t) -> p t d", d=DK)
                    for q in range(8):
                        psc = ps_c.tile([128, 512], F32, tag="psC")
                        for j in range(4):
                            for s8 in range(8):
                                tl = j * 64 + q * 8 + s8
                                nc.tensor.matmul(
                                    psc[32 * j : 32 * j + 16, s8 * 64 : (s8 + 1) * 64],
                                    lhsT=atv[:, tl, :],
                                    rhs=vtv[:, tl, :],
                                    start=True,
                                    stop=True,
                                    tile_position=(0, 32 * j),
                                )
                        nc.scalar.activation(
                            csb[:].rearrange("p (d tj) -> p tj d", d=DK)[
                                :, (w % 2) * 64 + q * 8 : (w % 2) * 64 + (q + 1) * 8, :
                            ],
                            psc[:].rearrange("p (s d) -> p s d", s=8),
                            Copy,
                        )
                    if w % 2 == 1:
                        # re-marshal this half-block to channel-major
                        # (baseline idiom: the 2-partition src rows merge
                        # with free d into dst partitions); emitting per
                        # half spreads the DMA burst across the block.
                        hb = w // 2
                        for j in range(4):
                            for k in range(8):
                                nc.sync.dma_start(
                                    cnats[b][:, :]
                                    .rearrange(
                                        "p (kk w j u) -> p kk w j u",
                                        kk=8,
                                        w=NW,
                                        j=4,
                                    )[:, k, 2 * hb : 2 * hb + 2, j, :],
                                    csb[
                                        32 * j + 2 * k : 32 * j + 2 * k + 2, :
                                    ].rearrange("p (d w u) -> p d w u", d=DK, w=2),
                                )

            def emit_outproj(b):
                cnat = cnats.pop(b)
                tB0 = b * SPC
                for mt in range(SPC // 128):
                    osb = opool.tile([128, C], BF16, tag="osb")
                    for n in range(2):
                        pso = ps_o.tile([128, 512], F32, tag="psO")
                        for k in range(8):
                            nc.tensor.matmul(
                                pso[:],
                                lhsT=cnat[
                                    :, k * SPC + mt * 128 : k * SPC + mt * 128 + 128
                                ],
                                rhs=wo_sb[k][:, n * 512 : (n + 1) * 512],
                                start=(k == 0),
                                stop=(k == 7),
                            )
                        evac2(osb[:, n * 512 : (n + 1) * 512], pso[:])
                    nc.sync.dma_start(
                        out_t[tB0 + mt * 128 : tB0 + mt * 128 + 128, :], osb[:]
                    )

            emit_norm(0)
            emit_norm(1)
            emit_norm(2)
            emit_vtload(0)
            for b in range(NBLK):
                emit_ctx(b)
                if b >= 1:
                    emit_outproj(b - 1)
            emit_outproj(NBLK - 1)

    _split_sync_waits(nc, limit=1)
    return nc


_NC_CACHE = {}


def _get_nc(T, SPAN):
    key = (T, SPAN)
    if key not in _NC_CACHE:
        _NC_CACHE[key] = build_kernel(T, SPAN)
    return _NC_CACHE[key]


def _prep_weights(w_qkv, b_qkv, w_out):
    bf = ml_dtypes.bfloat16
    w3 = w_qkv.reshape(H, 192, C).astype(np.float32)
    qw = (w3[:, :DK, :] / 8.0).reshape(H * DK, C)
    kw = w3[:, DK : 2 * DK, :].reshape(H * DK, C)
    vw = w3[:, 2 * DK :, :].reshape(H * DK, C)
    wqT = np.concatenate([qw, kw, vw], axis=0).T.copy().astype(bf)  # (C, 3072)
    b3 = b_qkv.reshape(H, 192).astype(np.float32)
    bq = np.concatenate(
        [(b3[:, :DK] / 8.0).reshape(-1), b3[:, DK : 2 * DK].reshape(-1), b3[:, 2 * DK :].reshape(-1)]
    )
    bqc = np.ascontiguousarray(bq.reshape(24, 128).T).astype(np.float32)  # (128, 24)
    woT = w_out.T.copy().astype(bf)  # (C, C) rows = (h,d) h-major
    return wqT, bqc, woT


def kernel(x, w_qkv, b_qkv, w_out, b_out, _trace=False, _span=256):
    B, _, T = x.shape
    assert B == N_CORES
    nc = _get_nc(T, _span)
    wqT, bqc, woT = _prep_weights(w_qkv, b_qkv, w_out)
    bf = ml_dtypes.bfloat16
    in_maps = []
    for b in range(B):
        in_maps.append(
            {
                "x": x[b].astype(bf),
                "wqT": wqT,
                "bqc": bqc,
                "woT": woT,
            }
        )
    res = run_bass_kernel_spmd(nc, in_maps, list(range(N_CORES)), trace=_trace)
    out = np.stack(
        [np.asarray(res.results[b]["outT"]).astype(np.float32).T for b in range(B)],
        axis=0,
    )
    out += b_out.astype(np.float32)[None, :, None]
    if _trace:
        kernel.last_exec_time_ns = res.exec_time_ns
        kernel.last_results = res
    return out


# revision 25
# speedup vs baseline: 1.0082x; 1.0082x over previous
"""Trainium2 Bass kernel for nn_MultiHeadAttention_53463752900838.

Math (per batch element b, one NeuronCore each — pure data parallel over B=8):
  qkv = w_qkv @ x + b_qkv                     (3072, T)
  q,k,v per head h: (64, T);  q scaled by 1/8 (folded into weights on host)
  scores[t,h,g] = sum_d q[h,d,t] k[g,d,t]     per-timestep 16x16 Gram matrix
  attn = softmax over t  (per (h,g) pair)
  context[h,d,t] = sum_g attn[t,h,g] v[g,d,t]
  out = w_out @ context + b_out               (1024, T)

Kernel layout (all bf16 matmuls, fp32 PSUM accumulation), software-pipelined
so the PE never idles:

  Pass 1 (per 256-t span s): project QKV; the PSUM evacuation adds b_qkv
    (activation Identity-with-bias / tensor_scalar add) and writes Q/K
    DIRECTLY into the scores layout qt/kt (64d, (h,t)) via two half-height
    evacs; V goes to a stage tile and is spilled+marshalled to DRAM in one
    strided DMA. Scores for span s-1 are emitted after the QKV matmuls of
    span s (PE queue stays full while evacs/DMAs of s-1 land); fused exp on
    ScalarE; running Z-reduce is delayed by 2 spans. exp(S) spills to DRAM.

  Pass 2 (per 512-t block): reload exp(S), normalize by 1/Z in place,
    per-t context matmuls with tile_position column tiling, one strided DMA
    re-marshals context to channel-major (cnat), final projection emitted one
    block behind so it overlaps the next block's context matmuls. Output is
    stored bf16 as out^T (t, o); host transposes and adds b_out.
"""

import os
import sys
import contextlib

import numpy as np
import ml_dtypes

for p in ("/opt/trn_rl_repo",):
    if p not in sys.path and os.path.isdir(p):
        sys.path.insert(0, p)

import concourse.bass as bass
import concourse.tile as tile
from concourse import mybir
from concourse.bass_utils import run_bass_kernel_spmd

F32 = mybir.dt.float32
BF16 = mybir.dt.bfloat16

N_CORES = 8
C = 1024
H = 16
DK = 64
OC3 = 3072


_WAITS2_OK = {
    "InstMatmult",
    "InstLdweights",
    "InstTensorCopy",
    "InstActivation",
    "InstTensorTensor",
    "InstTensorReduce",
    "InstDMACopy",
    "InstTensorScalarPtr",
    "InstMemset",
}


def _split_sync_waits(nc, limit=1):
    """walrus codegen rejects too many semaphore waits per instruction (CTRL
    class takes 1); hoist overflow waits onto NoOps inserted before the
    offending instruction. Compute/DMA instructions take 2."""
    counter = [0]
    n_split = 0
    for fn in nc.m.functions:
        for bb in fn.blocks:
            out = []
            for ins in bb.instructions:
                si = getattr(ins, "sync_info", None)
                waits = list(si.on_wait) if (si is not None and si.on_wait) else []
                if len(waits) > limit:
                    n_split += 1
                    extra, keep = waits[:-limit], waits[-limit:]
                    for i in range(0, len(extra), limit):
                        counter[0] += 1
                        out.append(
                            mybir.InstNoOp(
                                name=f"I-wsplit-{counter[0]}",
                                opcode="NoOp",
                                engine=ins.engine,
                                ins=[],
                                outs=[],
                                sync_info=mybir.SyncInfo(
                                    on_wait=list(extra[i : i + limit]), on_update=[]
                                ),
                            )
                        )
                    si.on_wait = keep
                out.append(ins)
            bb.instructions[:] = out
    return n_split


def build_kernel(T=4096, SPAN=256):
    """SPAN is kept for interface compat; pass-1 uses 512-t spans and pass-2
    uses 1024-t blocks of four 256-t sub-spans internally."""
    SP1 = 512  # pass-1 span
    NSP1 = T // SP1
    SS = 256  # pass-2 sub-span
    NW = 4  # sub-spans per pass-2 block
    SPC = NW * SS
    NBLK = T // SPC
    NSS = T // SS
    nc = bass.Bass("TRN2", target_bir_lowering=False, debug=False)

    x_in = nc.dram_tensor("x", [C, T], BF16, kind="ExternalInput").ap()
    wq_in = nc.dram_tensor("wqT", [C, OC3], BF16, kind="ExternalInput").ap()
    bq_in = nc.dram_tensor("bqc", [128, 24], F32, kind="ExternalInput").ap()
    wo_in = nc.dram_tensor("woT", [C, C], BF16, kind="ExternalInput").ap()
    out_t = nc.dram_tensor("outT", [T, C], BF16, kind="ExternalOutput").ap()
    # DRAM scratch: exp(scores) as (g, (h, t_abs)) and V as (g, (d, t_abs))
    se_d = nc.dram_tensor("se_d", [16, H * T], BF16).ap()
    vt_d = nc.dram_tensor("vt_d", [16, DK * T], BF16).ap()
    sev_d = se_d.rearrange("g (h t) -> g h t", h=H)
    vtv_d = vt_d.rearrange("(m hl) (d t) -> hl d m t", m=8, hl=2, d=DK)
    vbv_d = vt_d.rearrange("g (d t) -> g d t", d=DK)

    Exp = mybir.ActivationFunctionType.Exp
    Copy = mybir.ActivationFunctionType.Copy
    Ident = mybir.ActivationFunctionType.Identity
    ADD = mybir.AluOpType.add
    MUL = mybir.AluOpType.mult

    with tile.TileContext(nc) as tc, contextlib.ExitStack() as octx:
        const = octx.enter_context(tc.tile_pool(name="const", bufs=1))
        bqc = const.tile([128, 24], F32, tag="bqc")
        zacc = const.tile([16, 16], F32, tag="zacc")
        rrec = const.tile([16, 16], F32, tag="rrec")
        rrecb = const.tile([16, 16], BF16, tag="rrecb")
        wopool = octx.enter_context(tc.tile_pool(name="wo", bufs=1))
        wo_sb = []

        def emit_wo_chunk(k):
            w = wopool.tile([128, C], BF16, tag=f"wo{k}", name=f"wo{k}")
            nc.gpsimd.dma_start(w[:], wo_in[k * 128 : (k + 1) * 128, :])
            wo_sb.append(w)

        # ---------------- PASS 1 ----------------
        with contextlib.ExitStack() as ctx:
            wpool = ctx.enter_context(tc.tile_pool(name="wq", bufs=1))
            xpool = ctx.enter_context(tc.tile_pool(name="x", bufs=2))
            stpool = ctx.enter_context(tc.tile_pool(name="stage", bufs=2))
            qkpool = ctx.enter_context(tc.tile_pool(name="qkt", bufs=2))
            sepool = ctx.enter_context(tc.tile_pool(name="se", bufs=2))
            zpool = ctx.enter_context(tc.tile_pool(name="zp", bufs=2))
            ps_a = ctx.enter_context(tc.tile_pool(name="psA", bufs=4, space="PSUM"))
            ps_s = ctx.enter_context(tc.tile_pool(name="psS", bufs=3, space="PSUM"))

            xs = {}

            def emit_xload(s):
                xk = xpool.tile([128, 8 * SP1], BF16, tag="x")
                nc.sync.dma_start(
                    xk[:].rearrange("p (k t) -> p k t", k=8),
                    x_in[:, s * SP1 : (s + 1) * SP1].rearrange(
                        "(k p) t -> p k t", k=8
                    ),
                )
                xs[s] = xk

            # x span 0 + bias first so PE can start ASAP; wq chunks follow and
            # the inner-k matmul order paces with their arrival.
            nc.sync.dma_start(bqc[:], bq_in)
            emit_xload(0)
            wq_sb = []
            for k in range(8):
                w = wpool.tile([128, OC3], BF16, tag=f"wq{k}")
                nc.sync.dma_start(w[:], wq_in[k * 128 : (k + 1) * 128, :])
                wq_sb.append(w)

            qts, kts, ses = {}, {}, {}
            eng_tog = [0]

            def evac(dst, src, bias):
                """PSUM->SBUF evacuation with bias add, alternating engines."""
                eng_tog[0] ^= 1
                if eng_tog[0]:
                    nc.scalar.activation(dst, src, Ident, bias=bias)
                else:
                    nc.vector.tensor_scalar(dst, src, bias, None, ADD)

            NBLK1 = SP1 // 32  # scores blocks per span

            def emit_scores_block(s, blk, partial_zred=False):
                """One 32-t scores block (Gram matmuls + fused exp) of span s.
                On blk==0 allocates the span's se tile; on the last block
                spills exp(S) to DRAM."""
                if blk == 0:
                    ses[s] = sepool.tile([16, H * SP1], BF16, tag="se", name="se")
                se = ses[s]
                qtv = qts[s][:].rearrange("p (h t) -> p t h", h=H)
                ktv = kts[s][:].rearrange("p (g t) -> p t g", g=H)
                sev = se[:].rearrange("p (h t) -> p t h", h=H)
                pss = ps_s.tile([16, 512], F32, tag="psS")
                for s32 in range(32):
                    tl = blk * 32 + s32
                    nc.tensor.matmul(
                        pss[:, s32 * 16 : (s32 + 1) * 16],
                        lhsT=ktv[:, tl, :],
                        rhs=qtv[:, tl, :],
                        start=True,
                        stop=True,
                    )
                nc.scalar.activation(
                    sev[:, blk * 32 : (blk + 1) * 32, :],
                    pss[:].rearrange("p (t h) -> p t h", h=H),
                    Exp,
                )
                if partial_zred:
                    zp = zpool.tile([16, 16], F32, tag="zp")
                    nc.vector.tensor_reduce(
                        zp[:],
                        se[:].rearrange("p (h t) -> p h t", h=H)[
                            :, :, blk * 32 : (blk + 1) * 32
                        ],
                        axis=mybir.AxisListType.X,
                        op=ADD,
                    )
                    nc.vector.tensor_tensor(
                        out=zacc[:], in0=zacc[:], in1=zp[:], op=ADD
                    )
                if blk == NBLK1 - 1:
                    qts.pop(s)
                    kts.pop(s)
                    nc.gpsimd.dma_start(
                        sev_d[:, :, s * SP1 : (s + 1) * SP1],
                        se[:].rearrange("p (h t) -> p h t", h=H),
                    )

            def emit_qkv(s, sc=None):
                """QKV projection of span s; scores blocks of span sc (if any)
                are interleaved between the m-tiles so the PE never waits for
                the Act-paced exp evacuations."""
                xall = xs.pop(s)
                qt = qkpool.tile([64, H * SP1], BF16, tag="qt")
                kt = qkpool.tile([64, H * SP1], BF16, tag="kt")
                stage = stpool.tile([128, 8 * SP1], BF16, tag="st")
                qts[s], kts[s] = qt, kt
                for m in range(24):
                    kind, mm = divmod(m, 8)
                    ps = ps_a.tile([128, SP1], F32, tag="psA")
                    for k in range(8):
                        nc.tensor.matmul(
                            ps[:],
                            lhsT=wq_sb[k][:, m * 128 : (m + 1) * 128],
                            rhs=xall[:, k * SP1 : (k + 1) * SP1],
                            start=(k == 0),
                            stop=(k == 7),
                        )
                    if kind < 2:
                        dstt = qt if kind == 0 else kt
                        for hl in range(2):
                            h_abs = 2 * mm + hl
                            evac(
                                dstt[:, h_abs * SP1 : (h_abs + 1) * SP1],
                                ps[hl * 64 : (hl + 1) * 64, :],
                                bqc[hl * 64 : (hl + 1) * 64, m : m + 1],
                            )
                    else:
                        evac(
                            stage[:, mm * SP1 : (mm + 1) * SP1],
                            ps[:],
                            bqc[:, m : m + 1],
                        )
                    if sc is not None and m < NBLK1:
                        emit_scores_block(sc, m)
                # V spill+marshal: two strided DMAs,
                # SBUF (hl*64+d, (m,t)) -> DRAM (g=2m+hl, (d, t_abs))
                for hl in range(2):
                    nc.gpsimd.dma_start(
                        vtv_d[hl, :, :, s * SP1 : (s + 1) * SP1],
                        stage[hl * 64 : (hl + 1) * 64, :].rearrange(
                            "d (m t) -> d m t", m=8
                        ),
                    )

            def emit_zred(s):
                zp = zpool.tile([16, 16], F32, tag="zp")
                nc.vector.tensor_reduce(
                    zp[:],
                    ses.pop(s)[:].rearrange("p (h t) -> p h t", h=H),
                    axis=mybir.AxisListType.X,
                    op=ADD,
                )
                if s == 0:
                    nc.vector.tensor_copy(zacc[:], zp[:])
                else:
                    nc.vector.tensor_tensor(out=zacc[:], in0=zacc[:], in1=zp[:], op=ADD)

            for s in range(NSP1):
                emit_qkv(s, sc=s - 1 if s >= 1 else None)
                if s + 1 < NSP1:
                    emit_xload(s + 1)
                if s >= 2:
                    emit_zred(s - 2)
                # prefetch the pass-2 projection weights on the idle DMA
                # window mid-pass-1 (gpsimd queue; no WAR entanglement)
                if 2 <= s < 6:
                    emit_wo_chunk(2 * (s - 2))
                    emit_wo_chunk(2 * (s - 2) + 1)
            # last span's scores: emitted straight, with per-block partial
            # Z-reduces so the softmax denominator is ready ASAP after the
            # final exp (shortens the pass-1 -> pass-2 transition).
            emit_zred(NSP1 - 2)
            ls = NSP1 - 1
            for blk in range(NBLK1):
                emit_scores_block(ls, blk, partial_zred=True)
            ses.pop(ls)
            nc.vector.reciprocal(rrec[:], zacc[:])
            nc.vector.tensor_copy(rrecb[:], rrec[:])

        # ---------------- PASS 2 ----------------
        with contextlib.ExitStack() as ctx:
            sebpool = ctx.enter_context(tc.tile_pool(name="seb", bufs=2))
            atpool = ctx.enter_context(tc.tile_pool(name="atn", bufs=4))
            vtpool = ctx.enter_context(tc.tile_pool(name="vt2", bufs=2))
            cpool = ctx.enter_context(tc.tile_pool(name="csb", bufs=2))
            cnpool = ctx.enter_context(tc.tile_pool(name="cnat", bufs=2))
            opool = ctx.enter_context(tc.tile_pool(name="osb", bufs=2))
            ps_c = ctx.enter_context(tc.tile_pool(name="psC", bufs=4, space="PSUM"))
            ps_o = ctx.enter_context(tc.tile_pool(name="psO", bufs=3, space="PSUM"))

            rbc = rrecb[:].unsqueeze(2).broadcast_to([16, 16, SS])
            eng2 = [0]

            def evac2(dst, src):
                eng2[0] ^= 1
                if eng2[0]:
                    nc.scalar.activation(dst, src, Copy)
                else:
                    nc.vector.tensor_copy(dst, src)

            norm = {}

            def emit_norm(si):
                """Load + normalize exp(S) for 256-t sub-span si (prefetched).
                The normalize writes a separate tile (not in-place) so the
                load WAR chain does not couple into the multiply's latency,
                and alternates DVE / GpSimd so neither engine serializes."""
                seb = sebpool.tile([16, H * SS], BF16, tag="seb")
                nc.gpsimd.dma_start(
                    seb[:].rearrange("p (h t) -> p h t", h=H),
                    sev_d[:, :, si * SS : (si + 1) * SS],
                )
                atn = atpool.tile([16, H * SS], BF16, tag="atn")
                eng = nc.vector
                eng.tensor_tensor(
                    out=atn[:].rearrange("p (h t) -> p h t", h=H),
                    in0=seb[:].rearrange("p (h t) -> p h t", h=H),
                    in1=rbc,
                    op=MUL,
                )
                norm[si] = atn

            vts = {}

            def emit_vtload(si):
                vt = vtpool.tile([16, DK * SS], BF16, tag="vt2")
                nc.gpsimd.dma_start(
                    vt[:].rearrange("p (d t) -> p d t", d=DK),
                    vbv_d[:, :, si * SS : (si + 1) * SS],
                )
                vts[si] = vt

            cnats = {}

            def emit_ctx(b):
                cnats[b] = cnpool.tile([128, 8 * SPC], BF16, tag="cnat", name="cnat")
                csb = None
                for w in range(NW):
                    if w % 2 == 0:
                        csb = cpool.tile([128, DK * 128], BF16, tag="csb", name="csb")
                    si = b * NW + w
                    if si + 3 < NSS:
                        emit_norm(si + 3)
                    if si + 1 < NSS:
                        emit_vtload(si + 1)
                    atv = norm.pop(si)[:].rearrange("p (h t) -> p t h", h=H)
                    vtv = vts.pop(si)[:].rearrange("p (d t) -> p t d", d=DK)
                    for q in range(8):
                        psc = ps_c.tile([128, 512], F32, tag="psC")
                        for j in range(4):
                            for s8 in range(8):
                                tl = j * 64 + q * 8 + s8
                                nc.tensor.matmul(
                                    psc[32 * j : 32 * j + 16, s8 * 64 : (s8 + 1) * 64],
                                    lhsT=atv[:, tl, :],
                                    rhs=vtv[:, tl, :],
                                    start=True,
                                    stop=True,
                                    tile_position=(0, 32 * j),
                                )
                        nc.scalar.activation(
                            csb[:].rearrange("p (d tj) -> p tj d", d=DK)[
                                :, (w % 2) * 64 + q * 8 : (w % 2) * 64 + (q + 1) * 8, :
                            ],
                            psc[:].rearrange("p (s d) -> p s d", s=8),
                            Copy,
                        )
                    if w % 2 == 1:
                        # re-marshal this half-block to channel-major
                        # (baseline idiom: the 2-partition src rows merge
                        # with free d into dst partitions); emitting per
                        # half spreads the DMA burst across the block.
                        hb = w // 2
                        for j in range(4):
                            for k in range(8):
                                nc.sync.dma_start(
                                    cnats[b][:, :]
                                    .rearrange(
                                        "p (kk w j u) -> p kk w j u",
                                        kk=8,
                                        w=NW,
                                        j=4,
                                    )[:, k, 2 * hb : 2 * hb + 2, j, :],
                                    csb[
                                        32 * j + 2 * k : 32 * j + 2 * k + 2, :
                                    ].rearrange("p (d w u) -> p d w u", d=DK, w=2),
                                )

            def emit_outproj(b):
                cnat = cnats.pop(b)
                tB0 = b * SPC
                for mt in range(SPC // 128):
                    osb = opool.tile([128, C], BF16, tag="osb")
                    for n in range(2):
                        pso = ps_o.tile([128, 512], F32, tag="psO")
                        for k in range(8):
                            nc.tensor.matmul(
                                pso[:],
                                lhsT=cnat[
                                    :, k * SPC + mt * 128 : k * SPC + mt * 128 + 128
                                ],
                                rhs=wo_sb[k][:, n * 512 : (n + 1) * 512],
                                start=(k == 0),
                                stop=(k == 7),
                            )
                        evac2(osb[:, n * 512 : (n + 1) * 512], pso[:])
                    nc.sync.dma_start(
                        out_t[tB0 + mt * 128 : tB0 + mt * 128 + 128, :], osb[:]
                    )

            emit_norm(0)
            emit_norm(1)
            emit_norm(2)
            emit_vtload(0)
            for b in range(NBLK):
                emit_ctx(b)
                if b >= 1:
                    emit_outproj(b - 1)
            emit_outproj(NBLK - 1)

    _split_sync_waits(nc, limit=1)
    return nc


_NC_CACHE = {}


def _get_nc(T, SPAN):
    key = (T, SPAN)
    if key not in _NC_CACHE:
        _NC_CACHE[key] = build_kernel(T, SPAN)
    return _NC_CACHE[key]


def _prep_weights(w_qkv, b_qkv, w_out):
    bf = ml_dtypes.bfloat16
    w3 = w_qkv.reshape(H, 192, C).astype(np.float32)
    qw = (w3[:, :DK, :] / 8.0).reshape(H * DK, C)
    kw = w3[:, DK : 2 * DK, :].reshape(H * DK, C)
    vw = w3[:, 2 * DK :, :].reshape(H * DK, C)
    wqT = np.concatenate([qw, kw, vw], axis=0).T.copy().astype(bf)  # (C, 3072)
    b3 = b_qkv.reshape(H, 192).astype(np.float32)
    bq = np.concatenate(
        [(b3[:, :DK] / 8.0).reshape(-1), b3[:, DK : 2 * DK].reshape(-1), b3[:, 2 * DK :].reshape(-1)]
    )
    bqc = np.ascontiguousarray(bq.reshape(24, 128).T).astype(np.float32)  # (128, 24)
    woT = w_out.T.copy().astype(bf)  # (C, C) rows = (h,d) h-major
    return wqT, bqc, woT


def kernel(x, w_qkv, b_qkv, w_out, b_out, _trace=False, _span=256):
    B, _, T = x.shape
    assert B == N_CORES
    nc = _get_nc(T, _span)
    wqT, bqc, woT = _prep_weights(w_qkv, b_qkv, w_out)
    bf = ml_dtypes.bfloat16
    in_maps = []
    for b in range(B):
        in_maps.append(
            {
                "x": x[b].astype(bf),
                "wqT": wqT,
                "bqc": bqc,
                "woT": woT,
            }
        )
    res = run_bass_kernel_spmd(nc, in_maps, list(range(N_CORES)), trace=_trace)
    out = np.stack(
        [np.asarray(res.results[b]["outT"]).astype(np.float32).T for b in range(B)],
        axis=0,
    )
    out += b_out.astype(np.float32)[None, :, None]
    if _trace:
        kernel.last_exec_time_ns = res.exec_time_ns
        kernel.last_results = res
    return out


# revision 26
# speedup vs baseline: 1.0140x; 1.0057x over previous
"""Trainium2 Bass kernel for nn_MultiHeadAttention_53463752900838.

Math (per batch element b, one NeuronCore each — pure data parallel over B=8):
  qkv = w_qkv @ x + b_qkv                     (3072, T)
  q,k,v per head h: (64, T);  q scaled by 1/8 (folded into weights on host)
  scores[t,h,g] = sum_d q[h,d,t] k[g,d,t]     per-timestep 16x16 Gram matrix
  attn = softmax over t  (per (h,g) pair)
  context[h,d,t] = sum_g attn[t,h,g] v[g,d,t]
  out = w_out @ context + b_out               (1024, T)

Kernel layout (all bf16 matmuls, fp32 PSUM accumulation), software-pipelined
so the PE never idles:

  Pass 1 (per 256-t span s): project QKV; the PSUM evacuation adds b_qkv
    (activation Identity-with-bias / tensor_scalar add) and writes Q/K
    DIRECTLY into the scores layout qt/kt (64d, (h,t)) via two half-height
    evacs; V goes to a stage tile and is spilled+marshalled to DRAM in one
    strided DMA. Scores for span s-1 are emitted after the QKV matmuls of
    span s (PE queue stays full while evacs/DMAs of s-1 land); fused exp on
    ScalarE; running Z-reduce is delayed by 2 spans. exp(S) spills to DRAM.

  Pass 2 (per 512-t block): reload exp(S), normalize by 1/Z in place,
    per-t context matmuls with tile_position column tiling, one strided DMA
    re-marshals context to channel-major (cnat), final projection emitted one
    block behind so it overlaps the next block's context matmuls. Output is
    stored bf16 as out^T (t, o); host transposes and adds b_out.
"""

import os
import sys
import contextlib

import numpy as np
import ml_dtypes

for p in ("/opt/trn_rl_repo",):
    if p not in sys.path and os.path.isdir(p):
        sys.path.insert(0, p)

import concourse.bass as bass
import concourse.tile as tile
from concourse import mybir
from concourse.bass_utils import run_bass_kernel_spmd

F32 = mybir.dt.float32
BF16 = mybir.dt.bfloat16

N_CORES = 8
C = 1024
H = 16
DK = 64
OC3 = 3072


_WAITS2_OK = {
    "InstMatmult",
    "InstLdweights",
    "InstTensorCopy",
    "InstActivation",
    "InstTensorTensor",
    "InstTensorReduce",
    "InstDMACopy",
    "InstTensorScalarPtr",
    "InstMemset",
}


def _split_sync_waits(nc, limit=1):
    """walrus codegen rejects too many semaphore waits per instruction (CTRL
    class takes 1); hoist overflow waits onto NoOps inserted before the
    offending instruction. Compute/DMA instructions take 2."""
    counter = [0]
    n_split = 0
    for fn in nc.m.functions:
        for bb in fn.blocks:
            out = []
            for ins in bb.instructions:
                si = getattr(ins, "sync_info", None)
                waits = list(si.on_wait) if (si is not None and si.on_wait) else []
                if len(waits) > limit:
                    n_split += 1
                    extra, keep = waits[:-limit], waits[-limit:]
                    for i in range(0, len(extra), limit):
                        counter[0] += 1
                        out.append(
                            mybir.InstNoOp(
                                name=f"I-wsplit-{counter[0]}",
                                opcode="NoOp",
                                engine=ins.engine,
                                ins=[],
                                outs=[],
                                sync_info=mybir.SyncInfo(
                                    on_wait=list(extra[i : i + limit]), on_update=[]
                                ),
                            )
                        )
                    si.on_wait = keep
                out.append(ins)
            bb.instructions[:] = out
    return n_split


def build_kernel(T=4096, SPAN=256):
    """SPAN is kept for interface compat; pass-1 uses 512-t spans and pass-2
    uses 1024-t blocks of four 256-t sub-spans internally."""
    SP1 = 512  # pass-1 span
    NSP1 = T // SP1
    SS = 256  # pass-2 sub-span
    NW = 4  # sub-spans per pass-2 block
    SPC = NW * SS
    NBLK = T // SPC
    NSS = T // SS
    nc = bass.Bass("TRN2", target_bir_lowering=False, debug=False)

    x_in = nc.dram_tensor("x", [C, T], BF16, kind="ExternalInput").ap()
    wq_in = nc.dram_tensor("wqT", [C, OC3], BF16, kind="ExternalInput").ap()
    bq_in = nc.dram_tensor("bqc", [128, 24], F32, kind="ExternalInput").ap()
    wo_in = nc.dram_tensor("woT", [C, C], BF16, kind="ExternalInput").ap()
    out_t = nc.dram_tensor("outT", [T, C], BF16, kind="ExternalOutput").ap()
    # DRAM scratch: exp(scores) as (g, (h, t_abs)) and V as (g, (d, t_abs))
    se_d = nc.dram_tensor("se_d", [16, H * T], BF16).ap()
    vt_d = nc.dram_tensor("vt_d", [16, DK * T], BF16).ap()
    sev_d = se_d.rearrange("g (h t) -> g h t", h=H)
    vtv_d = vt_d.rearrange("(m hl) (d t) -> hl d m t", m=8, hl=2, d=DK)
    vbv_d = vt_d.rearrange("g (d t) -> g d t", d=DK)

    Exp = mybir.ActivationFunctionType.Exp
    Copy = mybir.ActivationFunctionType.Copy
    Ident = mybir.ActivationFunctionType.Identity
    ADD = mybir.AluOpType.add
    MUL = mybir.AluOpType.mult

    with tile.TileContext(nc) as tc, contextlib.ExitStack() as octx:
        const = octx.enter_context(tc.tile_pool(name="const", bufs=1))
        bqc = const.tile([128, 24], F32, tag="bqc")
        zacc = const.tile([16, 16], F32, tag="zacc")
        rrec = const.tile([16, 16], F32, tag="rrec")
        rrecb = const.tile([16, 16], BF16, tag="rrecb")
        wopool = octx.enter_context(tc.tile_pool(name="wo", bufs=1))
        wo_sb = []

        def emit_wo_chunk(k):
            w = wopool.tile([128, C], BF16, tag=f"wo{k}", name=f"wo{k}")
            nc.sync.dma_start(w[:], wo_in[k * 128 : (k + 1) * 128, :])
            wo_sb.append(w)

        # ---------------- PASS 1 ----------------
        with contextlib.ExitStack() as ctx:
            wpool = ctx.enter_context(tc.tile_pool(name="wq", bufs=1))
            xpool = ctx.enter_context(tc.tile_pool(name="x", bufs=2))
            stpool = ctx.enter_context(tc.tile_pool(name="stage", bufs=2))
            qkpool = ctx.enter_context(tc.tile_pool(name="qkt", bufs=2))
            sepool = ctx.enter_context(tc.tile_pool(name="se", bufs=2))
            zpool = ctx.enter_context(tc.tile_pool(name="zp", bufs=2))
            ps_a = ctx.enter_context(tc.tile_pool(name="psA", bufs=4, space="PSUM"))
            ps_s = ctx.enter_context(tc.tile_pool(name="psS", bufs=3, space="PSUM"))

            xs = {}

            def emit_xload(s):
                xk = xpool.tile([128, 8 * SP1], BF16, tag="x")
                nc.sync.dma_start(
                    xk[:].rearrange("p (k t) -> p k t", k=8),
                    x_in[:, s * SP1 : (s + 1) * SP1].rearrange(
                        "(k p) t -> p k t", k=8
                    ),
                )
                xs[s] = xk

            # x span 0 + bias first so PE can start ASAP; wq chunks follow and
            # the inner-k matmul order paces with their arrival.
            nc.sync.dma_start(bqc[:], bq_in)
            emit_xload(0)
            wq_sb = []
            for k in range(8):
                w = wpool.tile([128, OC3], BF16, tag=f"wq{k}")
                nc.sync.dma_start(w[:], wq_in[k * 128 : (k + 1) * 128, :])
                wq_sb.append(w)

            qts, kts, ses = {}, {}, {}
            eng_tog = [0]

            def evac(dst, src, bias):
                """PSUM->SBUF evacuation with bias add, alternating engines."""
                eng_tog[0] ^= 1
                if eng_tog[0]:
                    nc.scalar.activation(dst, src, Ident, bias=bias)
                else:
                    nc.vector.tensor_scalar(dst, src, bias, None, ADD)

            NBLK1 = SP1 // 32  # scores blocks per span

            def emit_scores_block(s, blk, partial_zred=False):
                """One 32-t scores block (Gram matmuls + fused exp) of span s.
                On blk==0 allocates the span's se tile; on the last block
                spills exp(S) to DRAM."""
                if blk == 0:
                    ses[s] = sepool.tile([16, H * SP1], BF16, tag="se", name="se")
                se = ses[s]
                qtv = qts[s][:].rearrange("p (h t) -> p t h", h=H)
                ktv = kts[s][:].rearrange("p (g t) -> p t g", g=H)
                sev = se[:].rearrange("p (h t) -> p t h", h=H)
                pss = ps_s.tile([16, 512], F32, tag="psS")
                for s32 in range(32):
                    tl = blk * 32 + s32
                    nc.tensor.matmul(
                        pss[:, s32 * 16 : (s32 + 1) * 16],
                        lhsT=ktv[:, tl, :],
                        rhs=qtv[:, tl, :],
                        start=True,
                        stop=True,
                    )
                nc.scalar.activation(
                    sev[:, blk * 32 : (blk + 1) * 32, :],
                    pss[:].rearrange("p (t h) -> p t h", h=H),
                    Exp,
                )
                if partial_zred:
                    zp = zpool.tile([16, 16], F32, tag="zp")
                    nc.vector.tensor_reduce(
                        zp[:],
                        se[:].rearrange("p (h t) -> p h t", h=H)[
                            :, :, blk * 32 : (blk + 1) * 32
                        ],
                        axis=mybir.AxisListType.X,
                        op=ADD,
                    )
                    nc.vector.tensor_tensor(
                        out=zacc[:], in0=zacc[:], in1=zp[:], op=ADD
                    )
                if blk == NBLK1 - 1:
                    qts.pop(s)
                    kts.pop(s)
                    nc.gpsimd.dma_start(
                        sev_d[:, :, s * SP1 : (s + 1) * SP1],
                        se[:].rearrange("p (h t) -> p h t", h=H),
                    )

            def emit_qkv(s, sc=None):
                """QKV projection of span s; scores blocks of span sc (if any)
                are interleaved between the m-tiles so the PE never waits for
                the Act-paced exp evacuations."""
                xall = xs.pop(s)
                qt = qkpool.tile([64, H * SP1], BF16, tag="qt")
                kt = qkpool.tile([64, H * SP1], BF16, tag="kt")
                stage = stpool.tile([128, 8 * SP1], BF16, tag="st")
                qts[s], kts[s] = qt, kt
                for m in range(24):
                    kind, mm = divmod(m, 8)
                    ps = ps_a.tile([128, SP1], F32, tag="psA")
                    for k in range(8):
                        nc.tensor.matmul(
                            ps[:],
                            lhsT=wq_sb[k][:, m * 128 : (m + 1) * 128],
                            rhs=xall[:, k * SP1 : (k + 1) * SP1],
                            start=(k == 0),
                            stop=(k == 7),
                        )
                    if kind < 2:
                        dstt = qt if kind == 0 else kt
                        for hl in range(2):
                            h_abs = 2 * mm + hl
                            evac(
                                dstt[:, h_abs * SP1 : (h_abs + 1) * SP1],
                                ps[hl * 64 : (hl + 1) * 64, :],
                                bqc[hl * 64 : (hl + 1) * 64, m : m + 1],
                            )
                    else:
                        evac(
                            stage[:, mm * SP1 : (mm + 1) * SP1],
                            ps[:],
                            bqc[:, m : m + 1],
                        )
                    if sc is not None and m < NBLK1:
                        emit_scores_block(sc, m)
                # V spill+marshal: two strided DMAs,
                # SBUF (hl*64+d, (m,t)) -> DRAM (g=2m+hl, (d, t_abs))
                for hl in range(2):
                    nc.gpsimd.dma_start(
                        vtv_d[hl, :, :, s * SP1 : (s + 1) * SP1],
                        stage[hl * 64 : (hl + 1) * 64, :].rearrange(
                            "d (m t) -> d m t", m=8
                        ),
                    )

            def emit_zred(s):
                zp = zpool.tile([16, 16], F32, tag="zp")
                nc.vector.tensor_reduce(
                    zp[:],
                    ses.pop(s)[:].rearrange("p (h t) -> p h t", h=H),
                    axis=mybir.AxisListType.X,
                    op=ADD,
                )
                if s == 0:
                    nc.vector.tensor_copy(zacc[:], zp[:])
                else:
                    nc.vector.tensor_tensor(out=zacc[:], in0=zacc[:], in1=zp[:], op=ADD)

            for s in range(NSP1):
                emit_qkv(s, sc=s - 1 if s >= 1 else None)
                if s + 1 < NSP1:
                    emit_xload(s + 1)
                if s >= 2:
                    emit_zred(s - 2)

            # last span's scores: emitted straight, with per-block partial
            # Z-reduces so the softmax denominator is ready ASAP after the
            # final exp (shortens the pass-1 -> pass-2 transition).
            emit_zred(NSP1 - 2)
            ls = NSP1 - 1
            for blk in range(NBLK1):
                emit_scores_block(ls, blk, partial_zred=True)
            ses.pop(ls)
            nc.vector.reciprocal(rrec[:], zacc[:])
            nc.vector.tensor_copy(rrecb[:], rrec[:])

        # ---------------- PASS 2 ----------------
        with contextlib.ExitStack() as ctx:
            sebpool = ctx.enter_context(tc.tile_pool(name="seb", bufs=2))
            atpool = ctx.enter_context(tc.tile_pool(name="atn", bufs=4))
            vtpool = ctx.enter_context(tc.tile_pool(name="vt2", bufs=2))
            cpool = ctx.enter_context(tc.tile_pool(name="csb", bufs=2))
            cnpool = ctx.enter_context(tc.tile_pool(name="cnat", bufs=2))
            opool = ctx.enter_context(tc.tile_pool(name="osb", bufs=2))
            ps_c = ctx.enter_context(tc.tile_pool(name="psC", bufs=4, space="PSUM"))
            ps_o = ctx.enter_context(tc.tile_pool(name="psO", bufs=3, space="PSUM"))

            rbc = rrecb[:].unsqueeze(2).broadcast_to([16, 16, SS])
            eng2 = [0]

            def evac2(dst, src):
                eng2[0] ^= 1
                if eng2[0]:
                    nc.scalar.activation(dst, src, Copy)
                else:
                    nc.vector.tensor_copy(dst, src)

            norm = {}

            def emit_norm(si):
                """Load + normalize exp(S) for 256-t sub-span si (prefetched).
                The normalize writes a separate tile (not in-place) so the
                load WAR chain does not couple into the multiply's latency,
                and alternates DVE / GpSimd so neither engine serializes."""
                seb = sebpool.tile([16, H * SS], BF16, tag="seb")
                nc.gpsimd.dma_start(
                    seb[:].rearrange("p (h t) -> p h t", h=H),
                    sev_d[:, :, si * SS : (si + 1) * SS],
                )
                atn = atpool.tile([16, H * SS], BF16, tag="atn")
                eng = nc.vector
                eng.tensor_tensor(
                    out=atn[:].rearrange("p (h t) -> p h t", h=H),
                    in0=seb[:].rearrange("p (h t) -> p h t", h=H),
                    in1=rbc,
                    op=MUL,
                )
                norm[si] = atn

            vts = {}

            def emit_vtload(si):
                vt = vtpool.tile([16, DK * SS], BF16, tag="vt2")
                nc.gpsimd.dma_start(
                    vt[:].rearrange("p (d t) -> p d t", d=DK),
                    vbv_d[:, :, si * SS : (si + 1) * SS],
                )
                vts[si] = vt

            cnats = {}

            def emit_ctx(b):
                cnats[b] = cnpool.tile([128, 8 * SPC], BF16, tag="cnat", name="cnat")
                csb = None
                for w in range(NW):
                    if w % 2 == 0:
                        csb = cpool.tile([128, DK * 128], BF16, tag="csb", name="csb")
                    si = b * NW + w
                    if si + 3 < NSS:
                        emit_norm(si + 3)
                    if si + 1 < NSS:
                        emit_vtload(si + 1)
                    atv = norm.pop(si)[:].rearrange("p (h t) -> p t h", h=H)
                    vtv = vts.pop(si)[:].rearrange("p (d t) -> p t d", d=DK)
                    for q in range(8):
                        psc = ps_c.tile([128, 512], F32, tag="psC")
                        for j in range(4):
                            for s8 in range(8):
                                tl = j * 64 + q * 8 + s8
                                nc.tensor.matmul(
                                    psc[32 * j : 32 * j + 16, s8 * 64 : (s8 + 1) * 64],
                                    lhsT=atv[:, tl, :],
                                    rhs=vtv[:, tl, :],
                                    start=True,
                                    stop=True,
                                    tile_position=(0, 32 * j),
                                )
                        nc.scalar.activation(
                            csb[:].rearrange("p (d tj) -> p tj d", d=DK)[
                                :, (w % 2) * 64 + q * 8 : (w % 2) * 64 + (q + 1) * 8, :
                            ],
                            psc[:].rearrange("p (s d) -> p s d", s=8),
                            Copy,
                        )
                    if w % 2 == 1:
                        # re-marshal this half-block to channel-major
                        # (baseline idiom: the 2-partition src rows merge
                        # with free d into dst partitions); emitting per
                        # half spreads the DMA burst across the block.
                        hb = w // 2
                        for j in range(4):
                            for k in range(8):
                                nc.sync.dma_start(
                                    cnats[b][:, :]
                                    .rearrange(
                                        "p (kk w j u) -> p kk w j u",
                                        kk=8,
                                        w=NW,
                                        j=4,
                                    )[:, k, 2 * hb : 2 * hb + 2, j, :],
                                    csb[
                                        32 * j + 2 * k : 32 * j + 2 * k + 2, :
                                    ].rearrange("p (d w u) -> p d w u", d=DK, w=2),
                                )

            def emit_outproj(b):
                cnat = cnats.pop(b)
                tB0 = b * SPC
                for mt in range(SPC // 128):
                    osb = opool.tile([128, C], BF16, tag="osb")
                    for n in range(2):
                        pso = ps_o.tile([128, 512], F32, tag="psO")
                        for k in range(8):
                            nc.tensor.matmul(
                                pso[:],
                                lhsT=cnat[
                                    :, k * SPC + mt * 128 : k * SPC + mt * 128 + 128
                                ],
                                rhs=wo_sb[k][:, n * 512 : (n + 1) * 512],
                                start=(k == 0),
                                stop=(k == 7),
                            )
                        evac2(osb[:, n * 512 : (n + 1) * 512], pso[:])
                    nc.sync.dma_start(
                        out_t[tB0 + mt * 128 : tB0 + mt * 128 + 128, :], osb[:]
                    )

            for k in range(8):
                emit_wo_chunk(k)
            emit_norm(0)
            emit_norm(1)
            emit_norm(2)
            emit_vtload(0)
            for b in range(NBLK):
                emit_ctx(b)
                if b >= 1:
                    emit_outproj(b - 1)
            emit_outproj(NBLK - 1)

    _split_sync_waits(nc, limit=1)
    return nc


_NC_CACHE = {}


def _get_nc(T, SPAN):
    key = (T, SPAN)
    if key not in _NC_CACHE:
        _NC_CACHE[key] = build_kernel(T, SPAN)
    return _NC_CACHE[key]


def _prep_weights(w_qkv, b_qkv, w_out):
    bf = ml_dtypes.bfloat16
    w3 = w_qkv.reshape(H, 192, C).astype(np.float32)
    qw = (w3[:, :DK, :] / 8.0).reshape(H * DK, C)
    kw = w3[:, DK : 2 * DK, :].reshape(H * DK, C)
    vw = w3[:, 2 * DK :, :].reshape(H * DK, C)
    wqT = np.concatenate([qw, kw, vw], axis=0).T.copy().astype(bf)  # (C, 3072)
    b3 = b_qkv.reshape(H, 192).astype(np.float32)
    bq = np.concatenate(
        [(b3[:, :DK] / 8.0).reshape(-1), b3[:, DK : 2 * DK].reshape(-1), b3[:, 2 * DK :].reshape(-1)]
    )
    bqc = np.ascontiguousarray(bq.reshape(24, 128).T).astype(np.float32)  # (128, 24)
    woT = w_out.T.copy().astype(bf)  # (C, C) rows = (h,d) h-major
    return wqT, bqc, woT


def kernel(x, w_qkv, b_qkv, w_out, b_out, _trace=False, _span=256):
    B, _, T = x.shape
    assert B == N_CORES
    nc = _get_nc(T, _span)
    wqT, bqc, woT = _prep_weights(w_qkv, b_qkv, w_out)
    bf = ml_dtypes.bfloat16
    in_maps = []
    for b in range(B):
        in_maps.append(
            {
                "x": x[b].astype(bf),
                "wqT": wqT,
                "bqc": bqc,
                "woT": woT,
            }
        )
    res = run_bass_kernel_spmd(nc, in_maps, list(range(N_CORES)), trace=_trace)
    out = np.stack(
        [np.asarray(res.results[b]["outT"]).astype(np.float32).T for b in range(B)],
        axis=0,
    )
    out += b_out.astype(np.float32)[None, :, None]
    if _trace:
        kernel.last_exec_time_ns = res.exec_time_ns
        kernel.last_results = res
    return out


# revision 28
# speedup vs baseline: 1.0140x; 1.0001x over previous
"""Trainium2 Bass kernel for nn_MultiHeadAttention_53463752900838.

Math (per batch element b, one NeuronCore each — pure data parallel over B=8):
  qkv = w_qkv @ x + b_qkv                     (3072, T)
  q,k,v per head h: (64, T);  q scaled by 1/8 (folded into weights on host)
  scores[t,h,g] = sum_d q[h,d,t] k[g,d,t]     per-timestep 16x16 Gram matrix
  attn = softmax over t  (per (h,g) pair)
  context[h,d,t] = sum_g attn[t,h,g] v[g,d,t]
  out = w_out @ context + b_out               (1024, T)

Kernel layout (all bf16 matmuls, fp32 PSUM accumulation), software-pipelined
so the PE never idles:

  Pass 1 (per 256-t span s): project QKV; the PSUM evacuation adds b_qkv
    (activation Identity-with-bias / tensor_scalar add) and writes Q/K
    DIRECTLY into the scores layout qt/kt (64d, (h,t)) via two half-height
    evacs; V goes to a stage tile and is spilled+marshalled to DRAM in one
    strided DMA. Scores for span s-1 are emitted after the QKV matmuls of
    span s (PE queue stays full while evacs/DMAs of s-1 land); fused exp on
    ScalarE; running Z-reduce is delayed by 2 spans. exp(S) spills to DRAM.

  Pass 2 (per 512-t block): reload exp(S), normalize by 1/Z in place,
    per-t context matmuls with tile_position column tiling, one strided DMA
    re-marshals context to channel-major (cnat), final projection emitted one
    block behind so it overlaps the next block's context matmuls. Output is
    stored bf16 as out^T (t, o); host transposes and adds b_out.
"""

import os
import sys
import contextlib

import numpy as np
import ml_dtypes

for p in ("/opt/trn_rl_repo",):
    if p not in sys.path and os.path.isdir(p):
        sys.path.insert(0, p)

import concourse.bass as bass
import concourse.tile as tile
from concourse import mybir
from concourse.bass_utils import run_bass_kernel_spmd

F32 = mybir.dt.float32
BF16 = mybir.dt.bfloat16

N_CORES = 8
C = 1024
H = 16
DK = 64
OC3 = 3072


_WAITS2_OK = {
    "InstMatmult",
    "InstLdweights",
    "InstTensorCopy",
    "InstActivation",
    "InstTensorTensor",
    "InstTensorReduce",
    "InstDMACopy",
    "InstTensorScalarPtr",
    "InstMemset",
}


def _split_sync_waits(nc, limit=1):
    """walrus codegen rejects too many semaphore waits per instruction (CTRL
    class takes 1); hoist overflow waits onto NoOps inserted before the
    offending instruction. Compute/DMA instructions take 2."""
    counter = [0]
    n_split = 0
    for fn in nc.m.functions:
        for bb in fn.blocks:
            out = []
            for ins in bb.instructions:
                si = getattr(ins, "sync_info", None)
                waits = list(si.on_wait) if (si is not None and si.on_wait) else []
                if len(waits) > limit:
                    n_split += 1
                    extra, keep = waits[:-limit], waits[-limit:]
                    for i in range(0, len(extra), limit):
                        counter[0] += 1
                        out.append(
                            mybir.InstNoOp(
                                name=f"I-wsplit-{counter[0]}",
                                opcode="NoOp",
                                engine=ins.engine,
                                ins=[],
                                outs=[],
                                sync_info=mybir.SyncInfo(
                                    on_wait=list(extra[i : i + limit]), on_update=[]
                                ),
                            )
                        )
                    si.on_wait = keep
                out.append(ins)
            bb.instructions[:] = out
    return n_split


def build_kernel(T=4096, SPAN=256):
    """SPAN is kept for interface compat; pass-1 uses 512-t spans and pass-2
    uses 1024-t blocks of four 256-t sub-spans internally."""
    SP1 = 512  # pass-1 span
    NSP1 = T // SP1
    SS = 256  # pass-2 sub-span
    NW = 4  # sub-spans per pass-2 block
    SPC = NW * SS
    NBLK = T // SPC
    NSS = T // SS
    nc = bass.Bass("TRN2", target_bir_lowering=False, debug=False)

    x_in = nc.dram_tensor("x", [C, T], BF16, kind="ExternalInput").ap()
    wq_in = nc.dram_tensor("wqT", [C, OC3], BF16, kind="ExternalInput").ap()
    bq_in = nc.dram_tensor("bqc", [128, 24], F32, kind="ExternalInput").ap()
    wo_in = nc.dram_tensor("woT", [C, C], BF16, kind="ExternalInput").ap()
    out_t = nc.dram_tensor("outT", [T, C], BF16, kind="ExternalOutput").ap()
    # DRAM scratch: exp(scores) as (g, (h, t_abs)) and V as (g, (d, t_abs))
    se_d = nc.dram_tensor("se_d", [16, H * T], BF16).ap()
    vt_d = nc.dram_tensor("vt_d", [16, DK * T], BF16).ap()
    sev_d = se_d.rearrange("g (h t) -> g h t", h=H)
    vtv_d = vt_d.rearrange("(m hl) (d t) -> hl d m t", m=8, hl=2, d=DK)
    vbv_d = vt_d.rearrange("g (d t) -> g d t", d=DK)

    Exp = mybir.ActivationFunctionType.Exp
    Copy = mybir.ActivationFunctionType.Copy
    Ident = mybir.ActivationFunctionType.Identity
    ADD = mybir.AluOpType.add
    MUL = mybir.AluOpType.mult

    with tile.TileContext(nc) as tc, contextlib.ExitStack() as octx:
        const = octx.enter_context(tc.tile_pool(name="const", bufs=1))
        bqc = const.tile([128, 24], F32, tag="bqc")
        zacc = const.tile([16, 16], F32, tag="zacc")
        rrec = const.tile([16, 16], F32, tag="rrec")
        rrecb = const.tile([16, 16], BF16, tag="rrecb")

        # ---------------- PASS 1 ----------------
        with contextlib.ExitStack() as ctx:
            wpool = ctx.enter_context(tc.tile_pool(name="wq", bufs=1))
            xpool = ctx.enter_context(tc.tile_pool(name="x", bufs=2))
            stpool = ctx.enter_context(tc.tile_pool(name="stage", bufs=2))
            qkpool = ctx.enter_context(tc.tile_pool(name="qkt", bufs=2))
            sepool = ctx.enter_context(tc.tile_pool(name="se", bufs=3))
            zpool = ctx.enter_context(tc.tile_pool(name="zp", bufs=2))
            ps_a = ctx.enter_context(tc.tile_pool(name="psA", bufs=4, space="PSUM"))
            ps_s = ctx.enter_context(tc.tile_pool(name="psS", bufs=3, space="PSUM"))

            xs = {}

            def emit_xload(s):
                xk = xpool.tile([128, 8 * SP1], BF16, tag="x")
                nc.sync.dma_start(
                    xk[:].rearrange("p (k t) -> p k t", k=8),
                    x_in[:, s * SP1 : (s + 1) * SP1].rearrange(
                        "(k p) t -> p k t", k=8
                    ),
                )
                xs[s] = xk

            # x span 0 + bias first so PE can start ASAP; wq chunks follow and
            # the inner-k matmul order paces with their arrival.
            nc.sync.dma_start(bqc[:], bq_in)
            emit_xload(0)
            wq_sb = []
            for k in range(8):
                w = wpool.tile([128, OC3], BF16, tag=f"wq{k}")
                nc.sync.dma_start(w[:], wq_in[k * 128 : (k + 1) * 128, :])
                wq_sb.append(w)

            qts, kts, ses = {}, {}, {}
            eng_tog = [0]

            def evac(dst, src, bias):
                """PSUM->SBUF evacuation with bias add, alternating engines."""
                eng_tog[0] ^= 1
                if eng_tog[0]:
                    nc.scalar.activation(dst, src, Ident, bias=bias)
                else:
                    nc.vector.tensor_scalar(dst, src, bias, None, ADD)

            NBLK1 = SP1 // 32  # scores blocks per span

            def emit_scores_block(s, blk, partial_zred=False):
                """One 32-t scores block (Gram matmuls + fused exp) of span s.
                On blk==0 allocates the span's se tile; on the last block
                spills exp(S) to DRAM."""
                if blk == 0:
                    ses[s] = sepool.tile([16, H * SP1], BF16, tag="se", name="se")
                se = ses[s]
                qtv = qts[s][:].rearrange("p (h t) -> p t h", h=H)
                ktv = kts[s][:].rearrange("p (g t) -> p t g", g=H)
                sev = se[:].rearrange("p (h t) -> p t h", h=H)
                pss = ps_s.tile([16, 512], F32, tag="psS")
                for s32 in range(32):
                    tl = blk * 32 + s32
                    nc.tensor.matmul(
                        pss[:, s32 * 16 : (s32 + 1) * 16],
                        lhsT=ktv[:, tl, :],
                        rhs=qtv[:, tl, :],
                        start=True,
                        stop=True,
                    )
                nc.scalar.activation(
                    sev[:, blk * 32 : (blk + 1) * 32, :],
                    pss[:].rearrange("p (t h) -> p t h", h=H),
                    Exp,
                )
                if partial_zred:
                    zp = zpool.tile([16, 16], F32, tag="zp")
                    nc.vector.tensor_reduce(
                        zp[:],
                        se[:].rearrange("p (h t) -> p h t", h=H)[
                            :, :, blk * 32 : (blk + 1) * 32
                        ],
                        axis=mybir.AxisListType.X,
                        op=ADD,
                    )
                    nc.vector.tensor_tensor(
                        out=zacc[:], in0=zacc[:], in1=zp[:], op=ADD
                    )
                if blk == NBLK1 - 1:
                    qts.pop(s)
                    kts.pop(s)
                    nc.gpsimd.dma_start(
                        sev_d[:, :, s * SP1 : (s + 1) * SP1],
                        se[:].rearrange("p (h t) -> p h t", h=H),
                    )

            def emit_qkv(s, sc=None):
                """QKV projection of span s; scores blocks of span sc (if any)
                are interleaved between the m-tiles so the PE never waits for
                the Act-paced exp evacuations."""
                xall = xs.pop(s)
                qt = qkpool.tile([64, H * SP1], BF16, tag="qt")
                kt = qkpool.tile([64, H * SP1], BF16, tag="kt")
                stage = stpool.tile([128, 8 * SP1], BF16, tag="st")
                qts[s], kts[s] = qt, kt
                for m in range(24):
                    kind, mm = divmod(m, 8)
                    ps = ps_a.tile([128, SP1], F32, tag="psA")
                    for k in range(8):
                        nc.tensor.matmul(
                            ps[:],
                            lhsT=wq_sb[k][:, m * 128 : (m + 1) * 128],
                            rhs=xall[:, k * SP1 : (k + 1) * SP1],
                            start=(k == 0),
                            stop=(k == 7),
                        )
                    if kind < 2:
                        dstt = qt if kind == 0 else kt
                        for hl in range(2):
                            h_abs = 2 * mm + hl
                            evac(
                                dstt[:, h_abs * SP1 : (h_abs + 1) * SP1],
                                ps[hl * 64 : (hl + 1) * 64, :],
                                bqc[hl * 64 : (hl + 1) * 64, m : m + 1],
                            )
                    else:
                        evac(
                            stage[:, mm * SP1 : (mm + 1) * SP1],
                            ps[:],
                            bqc[:, m : m + 1],
                        )
                    if sc is not None and m < NBLK1:
                        emit_scores_block(sc, m)
                # V spill+marshal: two strided DMAs,
                # SBUF (hl*64+d, (m,t)) -> DRAM (g=2m+hl, (d, t_abs))
                for hl in range(2):
                    nc.gpsimd.dma_start(
                        vtv_d[hl, :, :, s * SP1 : (s + 1) * SP1],
                        stage[hl * 64 : (hl + 1) * 64, :].rearrange(
                            "d (m t) -> d m t", m=8
                        ),
                    )

            def emit_zred(s):
                zp = zpool.tile([16, 16], F32, tag="zp")
                nc.vector.tensor_reduce(
                    zp[:],
                    ses.pop(s)[:].rearrange("p (h t) -> p h t", h=H),
                    axis=mybir.AxisListType.X,
                    op=ADD,
                )
                if s == 0:
                    nc.vector.tensor_copy(zacc[:], zp[:])
                else:
                    nc.vector.tensor_tensor(out=zacc[:], in0=zacc[:], in1=zp[:], op=ADD)

            for s in range(NSP1):
                emit_qkv(s, sc=s - 1 if s >= 1 else None)
                if s + 1 < NSP1:
                    emit_xload(s + 1)
                if s >= 2:
                    emit_zred(s - 2)

            # last span's scores: emitted straight, with per-block partial
            # Z-reduces so the softmax denominator is ready ASAP after the
            # final exp (shortens the pass-1 -> pass-2 transition).
            emit_zred(NSP1 - 2)
            ls = NSP1 - 1
            for blk in range(NBLK1):
                emit_scores_block(ls, blk, partial_zred=True)
            ses.pop(ls)
            nc.vector.reciprocal(rrec[:], zacc[:])
            nc.vector.tensor_copy(rrecb[:], rrec[:])

        # ---------------- PASS 2 ----------------
        with contextlib.ExitStack() as ctx:
            wopool = ctx.enter_context(tc.tile_pool(name="wo", bufs=1))
            wo_sb = []

            def emit_wo_chunk(k):
                w = wopool.tile([128, C], BF16, tag=f"wo{k}", name=f"wo{k}")
                nc.sync.dma_start(w[:], wo_in[k * 128 : (k + 1) * 128, :])
                wo_sb.append(w)

            sebpool = ctx.enter_context(tc.tile_pool(name="seb", bufs=2))
            atpool = ctx.enter_context(tc.tile_pool(name="atn", bufs=4))
            vtpool = ctx.enter_context(tc.tile_pool(name="vt2", bufs=2))
            cpool = ctx.enter_context(tc.tile_pool(name="csb", bufs=2))
            cnpool = ctx.enter_context(tc.tile_pool(name="cnat", bufs=2))
            opool = ctx.enter_context(tc.tile_pool(name="osb", bufs=2))
            ps_c = ctx.enter_context(tc.tile_pool(name="psC", bufs=4, space="PSUM"))
            ps_o = ctx.enter_context(tc.tile_pool(name="psO", bufs=3, space="PSUM"))

            rbc = rrecb[:].unsqueeze(2).broadcast_to([16, 16, SS])
            eng2 = [0]

            def evac2(dst, src):
                eng2[0] ^= 1
                if eng2[0]:
                    nc.scalar.activation(dst, src, Copy)
                else:
                    nc.vector.tensor_copy(dst, src)

            norm = {}

            def emit_norm(si):
                """Load + normalize exp(S) for 256-t sub-span si (prefetched).
                The normalize writes a separate tile (not in-place) so the
                load WAR chain does not couple into the multiply's latency,
                and alternates DVE / GpSimd so neither engine serializes."""
                seb = sebpool.tile([16, H * SS], BF16, tag="seb")
                nc.gpsimd.dma_start(
                    seb[:].rearrange("p (h t) -> p h t", h=H),
                    sev_d[:, :, si * SS : (si + 1) * SS],
                )
                atn = atpool.tile([16, H * SS], BF16, tag="atn")
                eng = nc.vector
                eng.tensor_tensor(
                    out=atn[:].rearrange("p (h t) -> p h t", h=H),
                    in0=seb[:].rearrange("p (h t) -> p h t", h=H),
                    in1=rbc,
                    op=MUL,
                )
                norm[si] = atn

            vts = {}

            def emit_vtload(si):
                vt = vtpool.tile([16, DK * SS], BF16, tag="vt2")
                nc.gpsimd.dma_start(
                    vt[:].rearrange("p (d t) -> p d t", d=DK),
                    vbv_d[:, :, si * SS : (si + 1) * SS],
                )
                vts[si] = vt

            cnats = {}

            def emit_ctx(b):
                cnats[b] = cnpool.tile([128, 8 * SPC], BF16, tag="cnat", name="cnat")
                csb = None
                for w in range(NW):
                    if w % 2 == 0:
                        csb = cpool.tile([128, DK * 128], BF16, tag="csb", name="csb")
                    si = b * NW + w
                    if si + 3 < NSS:
                        emit_norm(si + 3)
                    if si + 1 < NSS:
                        emit_vtload(si + 1)
                    atv = norm.pop(si)[:].rearrange("p (h t) -> p t h", h=H)
                    vtv = vts.pop(si)[:].rearrange("p (d t) -> p t d", d=DK)
                    for q in range(8):
                        psc = ps_c.tile([128, 512], F32, tag="psC")
                        for j in range(4):
                            for s8 in range(8):
                                tl = j * 64 + q * 8 + s8
                                nc.tensor.matmul(
                                    psc[32 * j : 32 * j + 16, s8 * 64 : (s8 + 1) * 64],
                                    lhsT=atv[:, tl, :],
                                    rhs=vtv[:, tl, :],
                                    start=True,
                                    stop=True,
                                    tile_position=(0, 32 * j),
                                )
                        nc.scalar.activation(
                            csb[:].rearrange("p (d tj) -> p tj d", d=DK)[
                                :, (w % 2) * 64 + q * 8 : (w % 2) * 64 + (q + 1) * 8, :
                            ],
                            psc[:].rearrange("p (s d) -> p s d", s=8),
                            Copy,
                        )
                    if w % 2 == 1:
                        # re-marshal this half-block to channel-major
                        # (baseline idiom: the 2-partition src rows merge
                        # with free d into dst partitions); emitting per
                        # half spreads the DMA burst across the block.
                        hb = w // 2
                        for j in range(4):
                            for k in range(8):
                                nc.sync.dma_start(
                                    cnats[b][:, :]
                                    .rearrange(
                                        "p (kk w j u) -> p kk w j u",
                                        kk=8,
                                        w=NW,
                                        j=4,
                                    )[:, k, 2 * hb : 2 * hb + 2, j, :],
                                    csb[
                                        32 * j + 2 * k : 32 * j + 2 * k + 2, :
                                    ].rearrange("p (d w u) -> p d w u", d=DK, w=2),
                                )

            def emit_outproj(b):
                cnat = cnats.pop(b)
                tB0 = b * SPC
                for mt in range(SPC // 128):
                    osb = opool.tile([128, C], BF16, tag="osb")
                    for n in range(2):
                        pso = ps_o.tile([128, 512], F32, tag="psO")
                        for k in range(8):
                            nc.tensor.matmul(
                                pso[:],
                                lhsT=cnat[
                                    :, k * SPC + mt * 128 : k * SPC + mt * 128 + 128
                                ],
                                rhs=wo_sb[k][:, n * 512 : (n + 1) * 512],
                                start=(k == 0),
                                stop=(k == 7),
                            )
                        evac2(osb[:, n * 512 : (n + 1) * 512], pso[:])
                    nc.sync.dma_start(
                        out_t[tB0 + mt * 128 : tB0 + mt * 128 + 128, :], osb[:]
                    )

            for k in range(8):
                emit_wo_chunk(k)
            emit_norm(0)
            emit_norm(1)
            emit_norm(2)
            emit_vtload(0)
            for b in range(NBLK):
                emit_ctx(b)
                if b >= 1:
                    emit_outproj(b - 1)
            emit_outproj(NBLK - 1)

    _split_sync_waits(nc, limit=1)
    return nc


_NC_CACHE = {}


def _get_nc(T, SPAN):
    key = (T, SPAN)
    if key not in _NC_CACHE:
        _NC_CACHE[key] = build_kernel(T, SPAN)
    return _NC_CACHE[key]


def _prep_weights(w_qkv, b_qkv, w_out):
    bf = ml_dtypes.bfloat16
    w3 = w_qkv.reshape(H, 192, C).astype(np.float32)
    qw = (w3[:, :DK, :] / 8.0).reshape(H * DK, C)
    kw = w3[:, DK : 2 * DK, :].reshape(H * DK, C)
    vw = w3[:, 2 * DK :, :].reshape(H * DK, C)
    wqT = np.concatenate([qw, kw, vw], axis=0).T.copy().astype(bf)  # (C, 3072)
    b3 = b_qkv.reshape(H, 192).astype(np.float32)
    bq = np.concatenate(
        [(b3[:, :DK] / 8.0).reshape(-1), b3[:, DK : 2 * DK].reshape(-1), b3[:, 2 * DK :].reshape(-1)]
    )
    bqc = np.ascontiguousarray(bq.reshape(24, 128).T).astype(np.float32)  # (128, 24)
    woT = w_out.T.copy().astype(bf)  # (C, C) rows = (h,d) h-major
    return wqT, bqc, woT


def kernel(x, w_qkv, b_qkv, w_out, b_out, _trace=False, _span=256):
    B, _, T = x.shape
    assert B == N_CORES
    nc = _get_nc(T, _span)
    wqT, bqc, woT = _prep_weights(w_qkv, b_qkv, w_out)
    bf = ml_dtypes.bfloat16
    in_maps = []
    for b in range(B):
        in_maps.append(
            {
                "x": x[b].astype(bf),
                "wqT": wqT,
                "bqc": bqc,
                "woT": woT,
            }
        )
    res = run_bass_kernel_spmd(nc, in_maps, list(range(N_CORES)), trace=_trace)
    out = np.stack(
        [np.asarray(res.results[b]["outT"]).astype(np.float32).T for b in range(B)],
        axis=0,
    )
    out += b_out.astype(np.float32)[None, :, None]
    if _trace:
        kernel.last_exec_time_ns = res.exec_time_ns
        kernel.last_results = res
    return out


# revision 30
# speedup vs baseline: 1.0858x; 1.0708x over previous
"""Trainium2 Bass kernel for nn_MultiHeadAttention_53463752900838.

Math (per batch element b, one NeuronCore each — pure data parallel over B=8):
  qkv = w_qkv @ x + b_qkv                     (3072, T)
  q,k,v per head h: (64, T);  q scaled by 1/8 (folded into weights on host)
  scores[t,h,g] = sum_d q[h,d,t] k[g,d,t]     per-timestep 16x16 Gram matrix
  attn = softmax over t  (per (h,g) pair)
  context[h,d,t] = sum_g attn[t,h,g] v[g,d,t]
  out = w_out @ context + b_out               (1024, T)

Kernel layout (all bf16 matmuls, fp32 PSUM accumulation), software-pipelined
so the PE never idles:

  Pass 1 (per 256-t span s): project QKV; the PSUM evacuation adds b_qkv
    (activation Identity-with-bias / tensor_scalar add) and writes Q/K
    DIRECTLY into the scores layout qt/kt (64d, (h,t)) via two half-height
    evacs; V goes to a stage tile and is spilled+marshalled to DRAM in one
    strided DMA. Scores for span s-1 are emitted after the QKV matmuls of
    span s (PE queue stays full while evacs/DMAs of s-1 land); fused exp on
    ScalarE; running Z-reduce is delayed by 2 spans. exp(S) spills to DRAM.

  Pass 2 (per 512-t block): reload exp(S), normalize by 1/Z in place,
    per-t context matmuls with tile_position column tiling, one strided DMA
    re-marshals context to channel-major (cnat), final projection emitted one
    block behind so it overlaps the next block's context matmuls. Output is
    stored bf16 as out^T (t, o); host transposes and adds b_out.
"""

import os
import sys
import contextlib

import numpy as np
import ml_dtypes

for p in ("/opt/trn_rl_repo",):
    if p not in sys.path and os.path.isdir(p):
        sys.path.insert(0, p)

import concourse.bass as bass
import concourse.tile as tile
from concourse import mybir
from concourse.bass_utils import run_bass_kernel_spmd

F32 = mybir.dt.float32
BF16 = mybir.dt.bfloat16

N_CORES = 8
C = 1024
H = 16
DK = 64
OC3 = 3072


_WAITS2_OK = {
    "InstMatmult",
    "InstLdweights",
    "InstTensorCopy",
    "InstActivation",
    "InstTensorTensor",
    "InstTensorReduce",
    "InstDMACopy",
    "InstTensorScalarPtr",
    "InstMemset",
}


def _split_sync_waits(nc, limit=1):
    """walrus codegen rejects too many semaphore waits per instruction (CTRL
    class takes 1); hoist overflow waits onto NoOps inserted before the
    offending instruction. Compute/DMA instructions take 2."""
    counter = [0]
    n_split = 0
    for fn in nc.m.functions:
        for bb in fn.blocks:
            out = []
            for ins in bb.instructions:
                si = getattr(ins, "sync_info", None)
                waits = list(si.on_wait) if (si is not None and si.on_wait) else []
                if len(waits) > limit:
                    n_split += 1
                    extra, keep = waits[:-limit], waits[-limit:]
                    for i in range(0, len(extra), limit):
                        counter[0] += 1
                        out.append(
                            mybir.InstNoOp(
                                name=f"I-wsplit-{counter[0]}",
                                opcode="NoOp",
                                engine=ins.engine,
                                ins=[],
                                outs=[],
                                sync_info=mybir.SyncInfo(
                                    on_wait=list(extra[i : i + limit]), on_update=[]
                                ),
                            )
                        )
                    si.on_wait = keep
                out.append(ins)
            bb.instructions[:] = out
    return n_split


def build_kernel(T=4096, SPAN=256):
    """SPAN is kept for interface compat; pass-1 uses 512-t spans and pass-2
    uses 1024-t blocks of four 256-t sub-spans internally."""
    SP1 = 512  # pass-1 span
    NSP1 = T // SP1
    SS = 256  # pass-2 sub-span
    NW = 4  # sub-spans per pass-2 block
    SPC = NW * SS
    NBLK = T // SPC
    NSS = T // SS
    nc = bass.Bass("TRN2", target_bir_lowering=False, debug=False)

    x_in = nc.dram_tensor("x", [C, T], BF16, kind="ExternalInput").ap()
    wq_in = nc.dram_tensor("wqT", [C, OC3], BF16, kind="ExternalInput").ap()
    bq_in = nc.dram_tensor("bqc", [128, 24], F32, kind="ExternalInput").ap()
    wo_in = nc.dram_tensor("woT", [C, C], BF16, kind="ExternalInput").ap()
    out_t = nc.dram_tensor("outT", [T, C], BF16, kind="ExternalOutput").ap()
    # DRAM scratch: exp(scores) as (g, (h, t_abs)) and V as (g, (d, t_abs))
    se_d = nc.dram_tensor("se_d", [16, H * T], BF16).ap()
    vt_d = nc.dram_tensor("vt_d", [16, DK * T], BF16).ap()
    sev_d = se_d.rearrange("g (h t) -> g h t", h=H)
    vtv_d = vt_d.rearrange("(m hl) (d t) -> hl d m t", m=8, hl=2, d=DK)
    vbv_d = vt_d.rearrange("g (d t) -> g d t", d=DK)

    Exp = mybir.ActivationFunctionType.Exp
    Copy = mybir.ActivationFunctionType.Copy
    Ident = mybir.ActivationFunctionType.Identity
    ADD = mybir.AluOpType.add
    MUL = mybir.AluOpType.mult

    with tile.TileContext(nc) as tc, contextlib.ExitStack() as octx:
        const = octx.enter_context(tc.tile_pool(name="const", bufs=1))
        bqc = const.tile([128, 24], F32, tag="bqc")
        zacc = const.tile([16, 16], F32, tag="zacc")
        rrec = const.tile([16, 16], F32, tag="rrec")
        rrecb = const.tile([16, 16], BF16, tag="rrecb")

        # ---------------- PASS 1 ----------------
        with contextlib.ExitStack() as ctx:
            wpool = ctx.enter_context(tc.tile_pool(name="wq", bufs=1))
            xpool = ctx.enter_context(tc.tile_pool(name="x", bufs=2))
            stpool = ctx.enter_context(tc.tile_pool(name="stage", bufs=2))
            qkpool = ctx.enter_context(tc.tile_pool(name="qkt", bufs=2))
            sepool = ctx.enter_context(tc.tile_pool(name="se", bufs=3))
            zpool = ctx.enter_context(tc.tile_pool(name="zp", bufs=2))
            ps_a = ctx.enter_context(tc.tile_pool(name="psA", bufs=4, space="PSUM"))
            ps_s = ctx.enter_context(tc.tile_pool(name="psS", bufs=3, space="PSUM"))

            xs = {}

            def emit_xload(s):
                xk = xpool.tile([128, 8 * SP1], BF16, tag="x")
                nc.sync.dma_start(
                    xk[:].rearrange("p (k t) -> p k t", k=8),
                    x_in[:, s * SP1 : (s + 1) * SP1].rearrange(
                        "(k p) t -> p k t", k=8
                    ),
                )
                xs[s] = xk

            # x span 0 + bias first so PE can start ASAP; wq chunks follow and
            # the inner-k matmul order paces with their arrival.
            nc.sync.dma_start(bqc[:], bq_in)
            emit_xload(0)
            wq_sb = []
            for k in range(8):
                w = wpool.tile([128, OC3], BF16, tag=f"wq{k}")
                nc.sync.dma_start(w[:], wq_in[k * 128 : (k + 1) * 128, :])
                wq_sb.append(w)

            qts, kts, ses = {}, {}, {}
            eng_tog = [0]

            def evac(dst, src, bias):
                """PSUM->SBUF evacuation with bias add, alternating engines."""
                eng_tog[0] ^= 1
                if eng_tog[0]:
                    nc.scalar.activation(dst, src, Ident, bias=bias)
                else:
                    nc.vector.tensor_scalar(dst, src, bias, None, ADD)

            NBLK1 = SP1 // 32  # scores blocks per span

            def emit_scores_block(s, blk, partial_zred=False):
                """One 32-t scores block (Gram matmuls + fused exp) of span s.
                On blk==0 allocates the span's se tile; on the last block
                spills exp(S) to DRAM."""
                if blk == 0:
                    ses[s] = sepool.tile([16, H * SP1], BF16, tag="se", name="se")
                se = ses[s]
                qtv = qts[s][:].rearrange("p (h t) -> p t h", h=H)
                ktv = kts[s][:].rearrange("p (g t) -> p t g", g=H)
                sev = se[:].rearrange("p (h t) -> p t h", h=H)
                pss = ps_s.tile([16, 512], F32, tag="psS")
                for s32 in range(32):
                    tl = blk * 32 + s32
                    nc.tensor.matmul(
                        pss[:, s32 * 16 : (s32 + 1) * 16],
                        lhsT=ktv[:, tl, :],
                        rhs=qtv[:, tl, :],
                        start=True,
                        stop=True,
                    )
                nc.scalar.activation(
                    sev[:, blk * 32 : (blk + 1) * 32, :],
                    pss[:].rearrange("p (t h) -> p t h", h=H),
                    Exp,
                )
                if partial_zred:
                    zp = zpool.tile([16, 16], F32, tag="zp")
                    nc.vector.tensor_reduce(
                        zp[:],
                        se[:].rearrange("p (h t) -> p h t", h=H)[
                            :, :, blk * 32 : (blk + 1) * 32
                        ],
                        axis=mybir.AxisListType.X,
                        op=ADD,
                    )
                    nc.vector.tensor_tensor(
                        out=zacc[:], in0=zacc[:], in1=zp[:], op=ADD
                    )
                if blk == NBLK1 - 1:
                    qts.pop(s)
                    kts.pop(s)
                    nc.gpsimd.dma_start(
                        sev_d[:, :, s * SP1 : (s + 1) * SP1],
                        se[:].rearrange("p (h t) -> p h t", h=H),
                    )

            def emit_qkv(s, sc=None):
                """QKV projection of span s; scores blocks of span sc (if any)
                are interleaved between the m-tiles so the PE never waits for
                the Act-paced exp evacuations."""
                xall = xs.pop(s)
                qt = qkpool.tile([64, H * SP1], BF16, tag="qt")
                kt = qkpool.tile([64, H * SP1], BF16, tag="kt")
                stage = stpool.tile([128, 8 * SP1], BF16, tag="st")
                qts[s], kts[s] = qt, kt
                for m in range(24):
                    kind, mm = divmod(m, 8)
                    ps = ps_a.tile([128, SP1], F32, tag="psA")
                    for k in range(8):
                        nc.tensor.matmul(
                            ps[:],
                            lhsT=wq_sb[k][:, m * 128 : (m + 1) * 128],
                            rhs=xall[:, k * SP1 : (k + 1) * SP1],
                            start=(k == 0),
                            stop=(k == 7),
                        )
                    if kind < 2:
                        dstt = qt if kind == 0 else kt
                        for hl in range(2):
                            h_abs = 2 * mm + hl
                            evac(
                                dstt[:, h_abs * SP1 : (h_abs + 1) * SP1],
                                ps[hl * 64 : (hl + 1) * 64, :],
                                bqc[hl * 64 : (hl + 1) * 64, m : m + 1],
                            )
                    else:
                        evac(
                            stage[:, mm * SP1 : (mm + 1) * SP1],
                            ps[:],
                            bqc[:, m : m + 1],
                        )
                    if sc is not None and m < NBLK1:
                        emit_scores_block(sc, m)
                # V spill+marshal: two strided DMAs,
                # SBUF (hl*64+d, (m,t)) -> DRAM (g=2m+hl, (d, t_abs))
                for hl in range(2):
                    nc.gpsimd.dma_start(
                        vtv_d[hl, :, :, s * SP1 : (s + 1) * SP1],
                        stage[hl * 64 : (hl + 1) * 64, :].rearrange(
                            "d (m t) -> d m t", m=8
                        ),
                    )

            def emit_zred(s):
                zp = zpool.tile([16, 16], F32, tag="zp")
                nc.vector.tensor_reduce(
                    zp[:],
                    ses.pop(s)[:].rearrange("p (h t) -> p h t", h=H),
                    axis=mybir.AxisListType.X,
                    op=ADD,
                )
                if s == 0:
                    nc.vector.tensor_copy(zacc[:], zp[:])
                else:
                    nc.vector.tensor_tensor(out=zacc[:], in0=zacc[:], in1=zp[:], op=ADD)

            for s in range(NSP1):
                emit_qkv(s, sc=s - 1 if s >= 1 else None)
                if s + 1 < NSP1:
                    emit_xload(s + 1)
                if s >= 2:
                    emit_zred(s - 2)

            # last span's scores: emitted straight, with per-block partial
            # Z-reduces so the softmax denominator is ready ASAP after the
            # final exp (shortens the pass-1 -> pass-2 transition).
            emit_zred(NSP1 - 2)
            ls = NSP1 - 1
            for blk in range(NBLK1):
                emit_scores_block(ls, blk, partial_zred=True)
            ses.pop(ls)
            nc.vector.reciprocal(rrec[:], zacc[:])
            nc.vector.tensor_copy(rrecb[:], rrec[:])

        # ---------------- PASS 2 ----------------
        with contextlib.ExitStack() as ctx:
            wopool = ctx.enter_context(tc.tile_pool(name="wo", bufs=1))
            wo_sb = []

            def emit_wo_chunk(k):
                w = wopool.tile([128, C], BF16, tag=f"wo{k}", name=f"wo{k}")
                nc.sync.dma_start(w[:], wo_in[k * 128 : (k + 1) * 128, :])
                wo_sb.append(w)

            sebpool = ctx.enter_context(tc.tile_pool(name="seb", bufs=3))
            vtpool = ctx.enter_context(tc.tile_pool(name="vt2", bufs=2))
            cpool = ctx.enter_context(tc.tile_pool(name="csb", bufs=2))
            cnpool = ctx.enter_context(tc.tile_pool(name="cnat", bufs=2))
            opool = ctx.enter_context(tc.tile_pool(name="osb", bufs=2))
            ps_c = ctx.enter_context(tc.tile_pool(name="psC", bufs=4, space="PSUM"))
            ps_o = ctx.enter_context(tc.tile_pool(name="psO", bufs=3, space="PSUM"))

            rbc = rrecb[:].unsqueeze(2).broadcast_to([16, 16, SS])
            eng2 = [0]

            def evac2(dst, src):
                eng2[0] ^= 1
                if eng2[0]:
                    nc.scalar.activation(dst, src, Copy)
                else:
                    nc.vector.tensor_copy(dst, src)

            norm = {}

            def emit_norm(si):
                """Load + normalize exp(S) for 256-t sub-span si (prefetched)."""
                seb = sebpool.tile([16, H * SS], BF16, tag="seb")
                nc.gpsimd.dma_start(
                    seb[:].rearrange("p (h t) -> p h t", h=H),
                    sev_d[:, :, si * SS : (si + 1) * SS],
                )
                sv = seb[:].rearrange("p (h t) -> p h t", h=H)
                nc.vector.tensor_tensor(out=sv, in0=sv, in1=rbc, op=MUL)
                norm[si] = seb

            vts = {}

            def emit_vtload(si):
                vt = vtpool.tile([16, DK * SS], BF16, tag="vt2")
                nc.gpsimd.dma_start(
                    vt[:].rearrange("p (d t) -> p d t", d=DK),
                    vbv_d[:, :, si * SS : (si + 1) * SS],
                )
                vts[si] = vt

            cnats = {}

            def emit_ctx(b):
                cnats[b] = cnpool.tile([128, 8 * SPC], BF16, tag="cnat", name="cnat")
                csb = cpool.tile([128, DK * 64 * NW], BF16, tag="csb", name="csb")
                for w in range(NW):
                    si = b * NW + w
                    if si + 2 < NSS:
                        emit_norm(si + 2)
                    if si + 1 < NSS:
                        emit_vtload(si + 1)
                    atv = norm.pop(si)[:].rearrange("p (h t) -> p t h", h=H)
                    vtv = vts.pop(si)[:].rearrange("p (d t) -> p t d", d=DK)
                    for q in range(8):
                        psc = ps_c.tile([128, 512], F32, tag="psC")
                        for j in range(4):
                            for s8 in range(8):
                                tl = j * 64 + q * 8 + s8
                                nc.tensor.matmul(
                                    psc[32 * j : 32 * j + 16, s8 * 64 : (s8 + 1) * 64],
                                    lhsT=atv[:, tl, :],
                                    rhs=vtv[:, tl, :],
                                    start=True,
                                    stop=True,
                                    tile_position=(0, 32 * j),
                                )
                        nc.scalar.activation(
                            csb[:].rearrange("p (d tj) -> p tj d", d=DK)[
                                :, w * 64 + q * 8 : w * 64 + (q + 1) * 8, :
                            ],
                            psc[:].rearrange("p (s d) -> p s d", s=8),
                            Copy,
                        )
                    if w == NW - 1:
                        for j in range(4):
                            for k in range(8):
                                nc.sync.dma_start(
                                    cnats[b][:, :]
                                    .rearrange(
                                        "p (kk w j u) -> p kk w j u",
                                        kk=8,
                                        w=NW,
                                        j=4,
                                    )[:, k, :, j, :],
                                    csb[
                                        32 * j + 2 * k : 32 * j + 2 * k + 2, :
                                    ].rearrange("p (d w u) -> p d w u", d=DK, w=NW),
                                )

            def emit_outproj(b):
                cnat = cnats.pop(b)
                tB0 = b * SPC
                for mt in range(SPC // 128):
                    osb = opool.tile([128, C], BF16, tag="osb")
                    for n in range(2):
                        pso = ps_o.tile([128, 512], F32, tag="psO")
                        for k in range(8):
                            nc.tensor.matmul(
                                pso[:],
                                lhsT=cnat[
                                    :, k * SPC + mt * 128 : k * SPC + mt * 128 + 128
                                ],
                                rhs=wo_sb[k][:, n * 512 : (n + 1) * 512],
                                start=(k == 0),
                                stop=(k == 7),
                            )
                        evac2(osb[:, n * 512 : (n + 1) * 512], pso[:])
                    nc.sync.dma_start(
                        out_t[tB0 + mt * 128 : tB0 + mt * 128 + 128, :], osb[:]
                    )

            for k in range(8):
                emit_wo_chunk(k)
            emit_norm(0)
            emit_norm(1)
            emit_vtload(0)
            for b in range(NBLK):
                emit_ctx(b)
                if b >= 1:
                    emit_outproj(b - 1)
            emit_outproj(NBLK - 1)

    _split_sync_waits(nc, limit=1)
    return nc


_NC_CACHE = {}


def _get_nc(T, SPAN):
    key = (T, SPAN)
    if key not in _NC_CACHE:
        _NC_CACHE[key] = build_kernel(T, SPAN)
    return _NC_CACHE[key]


def _prep_weights(w_qkv, b_qkv, w_out):
    bf = ml_dtypes.bfloat16
    w3 = w_qkv.reshape(H, 192, C).astype(np.float32)
    qw = (w3[:, :DK, :] / 8.0).reshape(H * DK, C)
    kw = w3[:, DK : 2 * DK, :].reshape(H * DK, C)
    vw = w3[:, 2 * DK :, :].reshape(H * DK, C)
    wqT = np.concatenate([qw, kw, vw], axis=0).T.copy().astype(bf)  # (C, 3072)
    b3 = b_qkv.reshape(H, 192).astype(np.float32)
    bq = np.concatenate(
        [(b3[:, :DK] / 8.0).reshape(-1), b3[:, DK : 2 * DK].reshape(-1), b3[:, 2 * DK :].reshape(-1)]
    )
    bqc = np.ascontiguousarray(bq.reshape(24, 128).T).astype(np.float32)  # (128, 24)
    woT = w_out.T.copy().astype(bf)  # (C, C) rows = (h,d) h-major
    return wqT, bqc, woT


def kernel(x, w_qkv, b_qkv, w_out, b_out, _trace=False, _span=256):
    B, _, T = x.shape
    assert B == N_CORES
    nc = _get_nc(T, _span)
    wqT, bqc, woT = _prep_weights(w_qkv, b_qkv, w_out)
    bf = ml_dtypes.bfloat16
    in_maps = []
    for b in range(B):
        in_maps.append(
            {
                "x": x[b].astype(bf),
                "wqT": wqT,
                "bqc": bqc,
                "woT": woT,
            }
        )
    res = run_bass_kernel_spmd(nc, in_maps, list(range(N_CORES)), trace=_trace)
    out = np.stack(
        [np.asarray(res.results[b]["outT"]).astype(np.float32).T for b in range(B)],
        axis=0,
    )
    out += b_out.astype(np.float32)[None, :, None]
    if _trace:
        kernel.last_exec_time_ns = res.exec_time_ns
        kernel.last_results = res
    return out
